# revision 15
# baseline (speedup 1.0000x reference)
"""4-layer GCN (ArithmeticCircuitGNN) on 8 Trainium2 NeuronCores.

Node-parallel, aggregation on the TENSOR engine: 12544-padded shard/core,
LN affine folded into weights on host.  Per GCN layer:
  AllGather(u) -> dma_gather(u[src]) in (chunk, dst-block) cells ->
  one-hot matmul S^T @ msg accumulated in PSUM per dst block, seeded with
  the self-loop term via an identity matmul -> evac (*dinv_dst, +bias,
  relu, +residual).  No DRAM scatter-add, no V planes.
Each AllGather is split in two shard-row halves: the first half fires
mid-way through the previous layer's block loop and hides under compute.
The one-hot S tiles (fp8, 1.0 entries) and the gather index table are
shared by all 4 layers (same graph).

v2: the two d_h=256 layers (uf2/uf3) carry fp8 messages on the wire and
through the gather + one-hot matmuls (self-loop seed stays bf16 via a
separate bf16 copy of u); relu+residual+mean-accum fused into one
scalar_tensor_tensor.

v3: software-pipelined block loop.  Each block's work is split into
pass A (aggregate + PSUM evac into h, accumulating LN row sums) and
pass B (LN stats -> z -> W matmul -> u stores), with pass B emitted
DELTA blocks behind pass A so every engine queue always holds ready
work (the in-order sequencers otherwise serialize the ~14-op
cross-engine chain per block).  LN stats are batched over block pairs
and u/vt/out DMAs cover two blocks each (dma_start issue on the sync
sequencer costs ~0.8us).

kernel(**inputs) takes FULL numpy inputs, returns FULL [100000,128] out.
"""

import os
import numpy as np
import ml_dtypes

import concourse.bass as bass
import concourse.bacc as bacc
import concourse.mybir as mybir
import concourse.tile as tile
from concourse.bass_utils import run_bass_kernel_spmd

BF16 = ml_dtypes.bfloat16
FP8 = ml_dtypes.float8_e4m3

N = 100000
E = 300000
NCORES = 8
NLOC = 12500
NPAD = 12544          # 98 * 128
NT = 98
HALF = NPAD // 2      # 6272 = 49 * 128 (collective split point)
NBH = NT // 2         # 49 blocks per half
NG = NPAD * NCORES    # 100352
NGH = HALF * NCORES   # 50176 rows per uf half-tensor
CHUNK = HALF * 4      # 25088 rows (4 ranks' halves) per gather chunk
NCHUNK = 4            # A0 A1 B0 B1
D_IN, D_H, D_OUT = 128, 256, 128
EPS = 1e-5
CALLG = 8             # groups per gather call (8*128 = 1024 idxs, ring cap)
NQ = 4                # SWDGE queues
VT_AHEAD = 2          # self-term DMA prefetch distance (in PAIRS)
DELTA = 3             # pass-B lag behind pass A, in blocks
GAH = 3               # gather emission lookahead, in blocks

F32 = mybir.dt.float32
BF = mybir.dt.bfloat16
I16 = mybir.dt.int16
F8 = mybir.dt.float8e4

SKIP_AGG = bool(int(os.environ.get("KERNEL_SKIP_AGG", "0")))
SKIP_CC = bool(int(os.environ.get("KERNEL_SKIP_CC", "0")))


# ---------------------------------------------------------------- host prep

def _wrap16(idx):
    """[M] -> [128, M//16]: position i -> (i%16, i//16), replicated x8."""
    M = len(idx)
    w = np.zeros((128, M // 16), dtype=np.int16)
    t = idx.reshape(M // 16, 16).T
    for g in range(8):
        w[g * 16:(g + 1) * 16, :] = t
    return w


def _prep_graph(edge_index):
    """Cell (chunk, dst-block) schedule for one-hot-matmul aggregation.

    Chunks: A0/A1 = first shard-halves of ranks 0-3 / 4-7, B0/B1 = second
    halves, matching the split AllGather output tensors ufA/ufB.
    Returns (gidx[8], S[8], calls, block_groups, need_call, M, Gtot, dinv);
    calls = [(chunk, row_off, n_rows), ...] in emission order;
    block_groups[b] = [(call_idx, slot, G), ...];  schedule uniform across
    cores (group counts maxed over cores)."""
    src = np.asarray(edge_index[0], dtype=np.int64)
    dst = np.asarray(edge_index[1], dtype=np.int64)
    deg = np.bincount(dst, minlength=N).astype(np.float64) + 1.0
    dinv = (1.0 / np.sqrt(deg)).astype(np.float32)

    r_arr = dst // NLOC
    dloc = dst - r_arr * NLOC
    b_arr = dloc // 128
    drow = dloc - b_arr * 128
    srank = src // NLOC
    spad = src % NLOC                      # row within shard (pad ignored)
    half = (spad >= HALF).astype(np.int64)
    quad = srank // 4
    c_arr = half * 2 + quad
    crow = (srank % 4) * HALF + spad - half * HALF

    counts = np.bincount(
        (r_arr * NCHUNK + c_arr) * NT + b_arr,
        minlength=NCORES * NCHUNK * NT).reshape(NCORES, NCHUNK, NT)
    k = -(-counts.max(axis=0) // 128)        # [NCHUNK, NT] groups per cell
    ngroups = k.sum(axis=1)                  # per chunk
    base = np.zeros((NCHUNK, NT), np.int64)  # group base within chunk
    base[:, 1:] = np.cumsum(k, axis=1)[:, :-1]

    ncalls = [-(-int(g) // CALLG) for g in ngroups]
    merged = []                              # (chunk, local_call)
    for i in range(max(ncalls)):
        for c in range(NCHUNK):
            if i < ncalls[c]:
                merged.append((c, i))
    calls = []
    cidx = {}
    goff = {}                                # (c, local_call) -> global G base
    off = 0
    for ci, (c, i) in enumerate(merged):
        g0 = i * CALLG
        ng = min(CALLG, int(ngroups[c]) - g0)
        calls.append((c, off, ng * 128))
        cidx[(c, i)] = ci
        goff[(c, i)] = off // 128
        off += ng * 128
    M = off
    Gtot = M // 128

    # gmap[c, g_loc] -> global group id
    gmap = np.zeros((NCHUNK, max(1, int(ngroups.max()))), np.int64)
    callof = np.zeros_like(gmap)
    for c in range(NCHUNK):
        for g in range(int(ngroups[c])):
            i = g // CALLG
            gmap[c, g] = goff[(c, i)] + g % CALLG
            callof[c, g] = cidx[(c, i)]

    block_groups = [[] for _ in range(NT)]
    need_call = np.full(NT, -1, np.int64)
    for b in range(NT):
        for c in range(NCHUNK):
            for j in range(int(k[c, b])):
                g = int(base[c, b]) + j
                ci = int(callof[c, g])
                block_groups[b].append((ci, g % CALLG, int(gmap[c, g])))
                need_call[b] = max(need_call[b], ci)

    gidx, Ss = [], []
    for r in range(NCORES):
        m = r_arr == r
        ec, eb = c_arr[m], b_arr[m]
        ecrow, edrow = crow[m], drow[m]
        cell = ec * NT + eb
        order = np.argsort(cell, kind="stable")
        cell_s = cell[order]
        starts = np.searchsorted(cell_s, np.arange(NCHUNK * NT))
        tpos = np.arange(len(cell_s)) - starts[cell_s]
        g_loc = base.reshape(-1)[cell_s] + tpos // 128
        G = gmap[cell_s // NT, g_loc]
        row = G * 128 + tpos % 128
        g_rows = np.zeros(M, np.int16)
        g_rows[row] = ecrow[order]
        S3 = np.zeros((128, Gtot, 128), FP8)
        S3[tpos % 128, G, edrow[order]] = 1.0
        gidx.append(_wrap16(g_rows))
        Ss.append(np.ascontiguousarray(S3.reshape(128, Gtot * 128)))
    return gidx, Ss, calls, block_groups, need_call, M, Gtot, dinv


def _rep(v, p=128):
    return np.ascontiguousarray(
        np.broadcast_to(np.asarray(v, np.float32), (p, len(v))))


MP_BUFS = 4


def _check_liveness(calls, block_groups, need_call):
    """Every matmul must read a msg tile within the last MP_BUFS of its
    chunk's pool tag, else pool rotation clobbers it.  Emission follows
    the GAH-block lookahead used by layer_loop."""
    emitted = {c: [] for c in range(NCHUNK)}
    ci = 0
    for b in range(NT):
        while ci <= need_call[min(b + GAH, NT - 1)]:
            emitted[calls[ci][0]].append(ci)
            ci += 1
        for (cj, slot, G) in block_groups[b]:
            c = calls[cj][0]
            assert cj in emitted[c][-MP_BUFS:], (
                f"block {b} reads call {cj} beyond pool depth "
                f"{emitted[c][-MP_BUFS - 2:]}")


# ---------------------------------------------------------------- builder

def _build(M, Gtot, calls, block_groups, need_call,
           use_brow1, use_brow2, use_fg, use_fb):
    _check_liveness(calls, block_groups, need_call)
    nc = bacc.Bacc(None, target_bir_lowering=False, num_swdge_queues=NQ)

    def param(name, shape, dt, out=False):
        return nc.declare_dram_parameter(name, shape, dt, isOutput=out)

    u0_own = param("u0_own", [NPAD, D_IN], BF)
    gidx = param("gidx", [128, M // 16], I16)
    S_p = param("S", [128, Gtot * 128], F8)
    dinv_p = param("dinv", [128, NT], F32)
    ident_p = param("ident", [128, 128], BF)
    w0_p = param("w0", [128, D_H], BF)
    w1_p = param("w1", [2, 128, D_H], BF)
    w2_p = param("w2", [2, 128, D_H], BF)
    w3_p = param("w3", [2, 128, D_OUT], BF)
    b0_p = param("b0r", [128, D_H], F32)
    b1_p = param("b1r", [128, D_H], F32)
    b2_p = param("b2r", [128, D_H], F32)
    b3_p = param("b3r", [128, D_OUT], F32)
    brow1_p = param("brow1r", [128, D_H], F32) if use_brow1 else None
    brow2_p = param("brow2r", [128, D_H], F32) if use_brow2 else None
    fg_p = param("fgr", [128, D_OUT], F32) if use_fg else None
    fb_p = param("fbr", [128, D_OUT], F32) if use_fb else None
    out_p = param("out", [NLOC, D_OUT], F32, out=True)

    ul0 = nc.dram_tensor("ul0", [NPAD, D_IN], BF)
    ul23_bf = nc.dram_tensor("ul23_bf", [NPAD, D_H], BF)   # self-term source
    ul23_f8 = nc.dram_tensor("ul23_f8", [NPAD, D_H], F8)   # wire source
    ul4 = nc.dram_tensor("ul4", [NPAD, D_OUT], BF)
    uf0 = [nc.dram_tensor(f"uf0{h}", [NGH, D_IN], BF, addr_space="Shared")
           for h in "AB"]
    uf2 = [nc.dram_tensor(f"uf2{h}", [NGH, D_H], F8, addr_space="Shared")
           for h in "AB"]
    uf3 = [nc.dram_tensor(f"uf3{h}", [NGH, D_H], F8, addr_space="Shared")
           for h in "AB"]
    uf4 = [nc.dram_tensor(f"uf4{h}", [NGH, D_OUT], BF, addr_space="Shared")
           for h in "AB"]

    AX = mybir.AxisListType.X
    AF = mybir.ActivationFunctionType
    OP = mybir.AluOpType

    with tile.TileContext(nc) as tc:
        with (
            tc.tile_pool(name="const", bufs=1) as cp,
            tc.tile_pool(name="hbuf", bufs=1) as hp,
            tc.tile_pool(name="work", bufs=5) as wp,
            tc.tile_pool(name="pair", bufs=4) as pp,
            tc.tile_pool(name="vtp", bufs=VT_AHEAD + 2) as vp,
            tc.tile_pool(name="small", bufs=6) as sp,
            tc.tile_pool(name="msg", bufs=MP_BUFS) as mp,
            tc.tile_pool(name="psT", bufs=2, space="PSUM") as pT,
            tc.tile_pool(name="psM", bufs=2, space="PSUM") as pM,
            tc.tile_pool(name="psA", bufs=4, space="PSUM") as pA,
        ):
            def cload(par, shape, dt):
                t = cp.tile(shape, dt, tag=par.name)
                nc.sync.dma_start(t[:], par[:])
                return t

            def allgather_half(ul, uf, h):
                if SKIP_CC:
                    return
                nc.gpsimd.collective_compute(
                    "AllGather", OP.bypass,
                    ins=[ul[h * HALF:(h + 1) * HALF, :].opt()],
                    outs=[uf[h][:].opt()],
                    replica_groups=[list(range(NCORES))],
                )

            # Layer-1 collectives first (input staged to ul0: collectives
            # can't read IO tensors); they overlap the constant loads below.
            nc.sync.dma_start(ul0[0:HALF, :], u0_own[0:HALF, :])
            allgather_half(ul0, uf0, 0)
            nc.sync.dma_start(ul0[HALF:NPAD, :], u0_own[HALF:NPAD, :])
            allgather_half(ul0, uf0, 1)

            gi = cload(gidx, [128, M // 16], I16)
            S_sb = cload(S_p, [128, Gtot * 128], F8)
            S_v = S_sb.rearrange("p (g d) -> p g d", d=128)
            dv = cload(dinv_p, [128, NT], F32)
            idn = cload(ident_p, [128, 128], BF)
            w0 = cload(w0_p, [128, D_H], BF)

            def wload(par, d):
                t = cp.tile([128, 2, d], BF, tag=par.name)
                nc.sync.dma_start(t[:], par.rearrange("k p d -> p k d"))
                return t

            w1 = wload(w1_p, D_H)
            w2 = wload(w2_p, D_H)
            w3 = wload(w3_p, D_OUT)
            b0 = cload(b0_p, [128, D_H], F32)
            b1 = cload(b1_p, [128, D_H], F32)
            b2 = cload(b2_p, [128, D_H], F32)
            b3 = cload(b3_p, [128, D_OUT], F32)
            brow1 = cload(brow1_p, [128, D_H], F32) if use_brow1 else None
            brow2 = cload(brow2_p, [128, D_H], F32) if use_brow2 else None
            fg = cload(fg_p, [128, D_OUT], F32) if use_fg else None
            fb = cload(fb_p, [128, D_OUT], F32) if use_fb else None

            h_sb = hp.tile([128, NT, D_H], BF)

            def r3(t, d):
                return t.rearrange("(n p) d -> p n d", p=128)

            def emit_gather(ci, uf, d, dt):
                """One gather call -> flat msg tile; returns [128,G,d] view."""
                (c, off, n) = calls[ci]
                t = mp.tile([128, CALLG * d], dt, tag=f"m{c}")
                tv = t.rearrange("p (g d) -> p g d", d=d)
                src = uf[c // 2][(c % 2) * CHUNK:(c % 2 + 1) * CHUNK, :]
                nc.gpsimd.dma_gather(
                    tv[:, : n // 128, :], src,
                    gi[:, off // 16:(off + n) // 16], n, n, d,
                    queue_num=ci % NQ,
                )
                return tv

            def agg_block(b, tiles, vt, d):
                """Self-term seed + one-hot matmuls for block b -> psum."""
                ps = pA.tile([128, d], F32, tag="agg")
                groups = [] if SKIP_AGG else block_groups[b]
                nc.tensor.matmul(ps[:], idn[:], vt[:],
                                 start=True, stop=not groups)
                for j, (ci, slot, G) in enumerate(groups):
                    nc.tensor.matmul(ps[:], S_v[:, G, :], tiles[ci][:, slot, :],
                                     start=False, stop=(j == len(groups) - 1))
                return ps

            def transpose_mm(z_bf, w, d_out, kchunks, evac):
                """z_bf [128, kchunks*128] -> mm = z^T @ w in PSUM.
                evac: 'S' or 'V' engine for the transpose evacuation."""
                mm = pM.tile([128, d_out], F32, tag="mm")
                zt_ps = pT.tile([128, kchunks, 128], BF, tag="zt_ps")
                for kk in range(kchunks):
                    nc.tensor.transpose(
                        zt_ps[:, kk, :], z_bf[:, kk * 128:(kk + 1) * 128],
                        idn[:])
                zt = wp.tile([128, kchunks, 128], BF, tag="zt")
                if evac == "S":
                    nc.scalar.activation(zt[:], zt_ps[:], AF.Copy)
                else:
                    nc.vector.tensor_scalar_mul(zt[:], zt_ps[:], 1.0)
                for kk in range(kchunks):
                    nc.tensor.matmul(mm[:], zt[:, kk, :],
                                     w[:, kk, :] if kchunks > 1 else w[:],
                                     start=(kk == 0), stop=(kk == kchunks - 1))
                return mm

            # ---------------- LN stats over a block pair ------------------
            def pair_stats(p, st):
                """Consumes st['sums'] [128,2]; fills negmu/s/negmu_s [128,2].
                Variance via Square+accum per block (bias = per-block
                negmu), then batched sqrt/recip/scale over the pair."""
                d = st["d"]
                negmu = sp.tile([128, 2], F32, tag="negmu")
                nc.vector.tensor_scalar_mul(negmu[:], st["sums"][:], -1.0 / d)
                ssq = sp.tile([128, 2], F32, tag="ssq")
                for k in (0, 1):
                    sq = wp.tile([128, d], BF, tag="sq")
                    nc.scalar.activation(sq[:], st["h"][k], AF.Square,
                                         bias=negmu[:, k:k + 1],
                                         accum_out=ssq[:, k:k + 1])
                varp = sp.tile([128, 2], F32, tag="varp")
                nc.vector.tensor_scalar(varp[:], ssq[:], 1.0 / d, EPS,
                                        OP.mult, OP.add)
                sd = sp.tile([128, 2], F32, tag="sd")
                nc.scalar.sqrt(sd[:], varp[:])
                rstd = sp.tile([128, 2], F32, tag="rstd")
                nc.vector.reciprocal(rstd[:], sd[:])
                st["negmu"] = negmu
                st["rstd"] = rstd
                return negmu, rstd

            def ln_scale_pair(p, st):
                """s = rstd*dinv, negmu_s = negmu*s for the pair."""
                s = sp.tile([128, 2], F32, tag="s")
                nc.vector.tensor_tensor(s[:], st["rstd"][:],
                                        dv[:, 2 * p:2 * p + 2], OP.mult)
                negmu_s = sp.tile([128, 2], F32, tag="negmu_s")
                nc.vector.tensor_tensor(negmu_s[:], st["negmu"][:], s[:],
                                        OP.mult)
                return s, negmu_s

            def store_u_pair(p, w, brow, ul_bf, zs):
                """z (bf16) per block -> mm -> u bf16+fp8, pair-batched DMA."""
                ubf = pp.tile([128, 2, D_H], BF, tag="ubf")
                u8 = pp.tile([128, 2, D_H], F8, tag="u8")
                for k in (0, 1):
                    mm = transpose_mm(zs[k], w, D_H, 2, evac="S")
                    if brow is not None:
                        b = 2 * p + k
                        nc.vector.scalar_tensor_tensor(
                            ubf[:, k, :], brow[:], dv[:, b:b + 1], mm[:],
                            OP.mult, OP.add)
                    else:
                        nc.vector.tensor_scalar_mul(ubf[:, k, :], mm[:], 1.0)
                    nc.vector.tensor_scalar_mul(u8[:, k, :], ubf[:, k, :],
                                                1.0)
                nc.sync.dma_start(r3(ul_bf, D_H)[:, 2 * p:2 * p + 2, :],
                                  ubf[:])
                nc.sync.dma_start(r3(ul23_f8, D_H)[:, 2 * p:2 * p + 2, :],
                                  u8[:])

            # ---------------- pipelined layer loop ------------------------
            def layer_loop(uf, d, dt, ul_self, passA, stage1, stage2,
                           mid_pair, mid_cc=None):
                """passA(b, ps): immediate PSUM evac (writes h/sums).
                stage1(p)/stage2(p): pass-B pair stages, stage1 lagging
                DELTA blocks behind pass A and stage2 one pair-tick
                behind stage1 (so stage2's PE work never waits on fresh
                stage1 results).  Stages are emitted oldest-first within
                an iteration to keep each engine queue head ready.
                mid_cc fires right after stage2(mid_pair)."""
                tiles = {}
                vts = {}
                ci = 0
                lag2 = DELTA + 2 if stage2 is not None else None

                def vt_dma(pv):
                    vt = vp.tile([128, 2, d], BF, tag="vt")
                    nc.sync.dma_start(vt[:],
                                      r3(ul_self, d)[:, 2 * pv:2 * pv + 2, :])
                    vts[pv] = vt

                def stage(fn, b, is_last):
                    if fn is not None and 0 <= b < NT and b % 2 == 1:
                        pr = b // 2
                        fn(pr)
                        if is_last and mid_cc is not None and pr == mid_pair:
                            mid_cc()

                for pv in range(min(VT_AHEAD, NBH)):
                    vt_dma(pv)
                for b in range(NT):
                    while ci <= need_call[min(b + GAH, NT - 1)]:
                        tiles[ci] = emit_gather(ci, uf, d, dt)
                        ci += 1
                    pv = b // 2 + VT_AHEAD
                    if b % 2 == 0 and pv < NBH:
                        vt_dma(pv)
                    if stage2 is not None:
                        stage(stage2, b - lag2, True)
                        stage(stage1, b - DELTA, False)
                    else:
                        stage(stage1, b - DELTA, True)
                    ps = agg_block(b, tiles, vts[b // 2][:, b % 2, :], d)
                    passA(b, ps)
                end = NT + (lag2 if stage2 is not None else DELTA)
                for b in range(NT, end):
                    if stage2 is not None:
                        stage(stage2, b - lag2, True)
                        stage(stage1, b - DELTA, False)
                    else:
                        stage(stage1, b - DELTA, True)

            # ================= Layer 1 ====================================
            l1_st = {}

            def l1_passA(b, ps):
                p = b // 2
                if b % 2 == 0:
                    l1_st[p] = {"d": D_H,
                                "tbf": pp.tile([128, 2, D_IN], BF,
                                               tag="tbf", name="tbf")}
                nc.scalar.activation(l1_st[p]["tbf"][:, b % 2, :], ps[:],
                                     AF.Copy)

            l1_zs = {}

            def l1_stage1(p):
                st = l1_st.pop(p)
                st["sums"] = sp.tile([128, 2], F32, tag="sums",
                                     name="sums")
                st["h"] = [h_sb[:, 2 * p, :], h_sb[:, 2 * p + 1, :]]
                for k in (0, 1):
                    b = 2 * p + k
                    mm = transpose_mm(st["tbf"][:, k, :], w0, D_H, 1,
                                      evac="V")
                    t2 = wp.tile([128, D_H], F32, tag="t2")
                    nc.vector.scalar_tensor_tensor(
                        t2[:], mm[:], dv[:, b:b + 1], b0[:], OP.mult, OP.add)
                    # relu with row-sum accum, on vector (op1 bypass)
                    nc.vector.scalar_tensor_tensor(
                        h_sb[:, b, :], t2[:], 0.0, t2[:], OP.max, OP.bypass,
                        accum_out=st["sums"][:, k:k + 1])
                pair_stats(p, st)
                s, negmu_s = ln_scale_pair(p, st)
                zs = []
                for k in (0, 1):
                    z = wp.tile([128, D_H], BF, tag="z")
                    nc.scalar.activation(z[:], st["h"][k], AF.Identity,
                                         bias=negmu_s[:, k:k + 1],
                                         scale=s[:, k:k + 1])
                    zs.append(z)
                l1_zs[p] = zs

            def l1_stage2(p):
                store_u_pair(p, w1, brow1, ul23_bf, l1_zs.pop(p))

            layer_loop(uf0, D_IN, BF, u0_own, l1_passA, l1_stage1,
                       l1_stage2, mid_pair=NBH // 2,
                       mid_cc=lambda: allgather_half(ul23_f8, uf2, 0))
            allgather_half(ul23_f8, uf2, 1)

            # ================= Layers 2, 3 ================================
            def mk_mid(bias, st_map):
                def passA(b, ps):
                    p = b // 2
                    if b % 2 == 0:
                        st_map[p] = {"d": D_H,
                                     "sums": sp.tile([128, 2], F32,
                                                     tag="sums",
                                                     name="sums"),
                                     "h": [h_sb[:, 2 * p, :],
                                           h_sb[:, 2 * p + 1, :]]}
                    st = st_map[p]
                    t4 = wp.tile([128, D_H], F32, tag="t4")
                    nc.vector.scalar_tensor_tensor(
                        t4[:], ps[:], dv[:, b:b + 1], bias[:],
                        OP.mult, OP.add)
                    # h = relu(t4) + h_old, accumulating LN row sums
                    nc.vector.scalar_tensor_tensor(
                        h_sb[:, b, :], t4[:], 0.0, h_sb[:, b, :],
                        OP.max, OP.add, accum_out=st["sums"][:, b % 2:
                                                             b % 2 + 1])
                return passA

            def mk_mid_stage1(st_map, zs_map):
                def stage1(p):
                    st = st_map.pop(p)
                    pair_stats(p, st)
                    s, negmu_s = ln_scale_pair(p, st)
                    zs = []
                    for k in (0, 1):
                        z = wp.tile([128, D_H], BF, tag="z")
                        nc.scalar.activation(z[:], st["h"][k], AF.Identity,
                                             bias=negmu_s[:, k:k + 1],
                                             scale=s[:, k:k + 1])
                        zs.append(z)
                    zs_map[p] = zs
                return stage1

            def mk_mid_stage2(zs_map, w, brow):
                def stage2(p):
                    store_u_pair(p, w, brow, ul23_bf, zs_map.pop(p))
                return stage2

            l2_st, l2_zs = {}, {}
            layer_loop(uf2, D_H, F8, ul23_bf, mk_mid(b1, l2_st),
                       mk_mid_stage1(l2_st, l2_zs),
                       mk_mid_stage2(l2_zs, w2, brow2),
                       mid_pair=NBH // 2,
                       mid_cc=lambda: allgather_half(ul23_f8, uf3, 0))
            allgather_half(ul23_f8, uf3, 1)

            l3_st, l3_zs = {}, {}

            def l3_stage1(p):
                # u-compute for layer 4: (h * dinv) @ W3  (no LN)
                st = l3_st.pop(p)
                zs = []
                for k in (0, 1):
                    b = 2 * p + k
                    z = wp.tile([128, D_H], BF, tag="z")
                    nc.vector.tensor_scalar_mul(z[:], st["h"][k],
                                                dv[:, b:b + 1])
                    zs.append(z)
                l3_zs[p] = zs

            def l3_stage2(p):
                zs = l3_zs.pop(p)
                ubf = pp.tile([128, 2, D_OUT], BF, tag="ub4")
                for k in (0, 1):
                    mm = transpose_mm(zs[k], w3, D_OUT, 2, evac="S")
                    nc.scalar.activation(ubf[:, k, :], mm[:], AF.Copy)
                nc.sync.dma_start(r3(ul4, D_OUT)[:, 2 * p:2 * p + 2, :],
                                  ubf[:])

            layer_loop(uf3, D_H, F8, ul23_bf, mk_mid(b2, l3_st), l3_stage1,
                       l3_stage2, mid_pair=NBH // 2,
                       mid_cc=lambda: allgather_half(ul4, uf4, 0))
            allgather_half(ul4, uf4, 1)

            # ================= Layer 4 ====================================
            l4_st = {}

            def l4_passA(b, ps):
                p = b // 2
                if b % 2 == 0:
                    l4_st[p] = {"d": D_OUT,
                                "sums": sp.tile([128, 2], F32, tag="sums",
                                                name="sums"),
                                "y2": pp.tile([128, 2, D_OUT], F32,
                                              tag="y2", name="y2")}
                st = l4_st[p]
                nc.vector.scalar_tensor_tensor(
                    st["y2"][:, b % 2, :], ps[:], dv[:, b:b + 1], b3[:],
                    OP.mult, OP.add, accum_out=st["sums"][:, b % 2:b % 2 + 1])

            def l4_workP(p):
                st = l4_st.pop(p)
                st["h"] = [st["y2"][:, 0, :], st["y2"][:, 1, :]]
                pair_stats(p, st)
                zo = pp.tile([128, 2, D_OUT], F32, tag="zo")
                for k in (0, 1):
                    nc.vector.tensor_scalar(
                        zo[:, k, :], st["y2"][:, k, :],
                        st["negmu"][:, k:k + 1], st["rstd"][:, k:k + 1],
                        OP.add, OP.mult)
                    if fg is not None:
                        nc.vector.tensor_tensor(zo[:, k, :], zo[:, k, :],
                                                fg[:], OP.mult)
                    if fb is not None:
                        nc.vector.tensor_tensor(zo[:, k, :], zo[:, k, :],
                                                fb[:], OP.add)
                lo = 2 * p * 128
                if lo + 256 <= NLOC:
                    nc.sync.dma_start(
                        out_p[lo:lo + 256, :].rearrange(
                            "(n p) d -> p n d", p=128), zo[:])
                else:
                    for k in (0, 1):
                        l2_ = lo + k * 128
                        nrow = min(128, NLOC - l2_)
                        if nrow > 0:
                            nc.sync.dma_start(out_p[l2_:l2_ + nrow, :],
                                              zo[0:nrow, k, :])

            layer_loop(uf4, D_OUT, BF, ul4, l4_passA, l4_workP, None,
                       mid_pair=10**9)

    nc.compile()
    return nc


_CACHE = {}


def kernel(x, edge_index, W0, b0, W1, b1, W2, b2, W3, b3,
           ln0_g, ln0_b, ln1_g, ln1_b, fln_g, fln_b):
    x = np.asarray(x, np.float32)
    edge_index = np.asarray(edge_index)
    (gidx, Ss, calls, block_groups, need_call, M, Gtot,
     dinv) = _prep_graph(edge_index)

    W1f = np.asarray(ln0_g, np.float32)[:, None] * np.asarray(W1, np.float32)
    W2f = np.asarray(ln1_g, np.float32)[:, None] * np.asarray(W2, np.float32)
    brow1 = np.asarray(ln0_b, np.float32) @ np.asarray(W1, np.float32)
    brow2 = np.asarray(ln1_b, np.float32) @ np.asarray(W2, np.float32)
    use_brow1 = bool(np.any(brow1 != 0))
    use_brow2 = bool(np.any(brow2 != 0))
    use_fg = bool(np.any(np.asarray(fln_g) != 1))
    use_fb = bool(np.any(np.asarray(fln_b) != 0))

    key = (M, Gtot, tuple(calls), tuple(need_call),
           tuple(tuple(g) for g in block_groups),
           use_brow1, use_brow2, use_fg, use_fb)
    if key not in _CACHE:
        _CACHE[key] = _build(M, Gtot, calls, block_groups, need_call,
                             use_brow1, use_brow2, use_fg, use_fb)
    nc = _CACHE[key]

    u0 = dinv[:, None].astype(np.float32) * x
    u0p = np.zeros((NCORES, NPAD, D_IN), BF16)
    for r in range(NCORES):
        u0p[r, :NLOC] = u0[r * NLOC:(r + 1) * NLOC]
    dinv_pad = np.zeros((NCORES, NPAD), np.float32)
    for r in range(NCORES):
        dinv_pad[r, :NLOC] = dinv[r * NLOC:(r + 1) * NLOC]

    def chunk2(Wf):
        return np.stack([Wf[0:128], Wf[128:256]]).astype(BF16)

    common = {
        "ident": np.eye(128, dtype=BF16),
        "w0": np.asarray(W0, np.float32).astype(BF16),
        "w1": chunk2(W1f), "w2": chunk2(W2f),
        "w3": chunk2(np.asarray(W3, np.float32)),
        "b0r": _rep(b0), "b1r": _rep(b1), "b2r": _rep(b2), "b3r": _rep(b3),
    }
    if use_brow1:
        common["brow1r"] = _rep(brow1)
    if use_brow2:
        common["brow2r"] = _rep(brow2)
    if use_fg:
        common["fgr"] = _rep(fln_g)
    if use_fb:
        common["fbr"] = _rep(fln_b)

    in_maps = []
    for r in range(NCORES):
        m = dict(common)
        m["u0_own"] = u0p[r]
        m["gidx"] = gidx[r]
        m["S"] = Ss[r]
        m["dinv"] = np.ascontiguousarray(dinv_pad[r].reshape(NT, 128).T)
        in_maps.append(m)

    res = run_bass_kernel_spmd(nc, in_maps, core_ids=list(range(NCORES)))
    out = np.concatenate([res.results[r]["out"] for r in range(NCORES)],
                         axis=0)
    return out.astype(np.float32)


# revision 16
# speedup vs baseline: 1.0204x; 1.0204x over previous
"""4-layer GCN (ArithmeticCircuitGNN) on 8 Trainium2 NeuronCores.

Node-parallel, aggregation on the TENSOR engine: 12544-padded shard/core,
LN affine folded into weights on host.  Per GCN layer:
  AllGather(u) -> dma_gather(u[src]) in (chunk, dst-block) cells ->
  one-hot matmul S^T @ msg accumulated in PSUM per dst block, seeded with
  the self-loop term via an identity matmul -> evac (*dinv_dst, +bias,
  relu, +residual).  No DRAM scatter-add, no V planes.
Each AllGather is split in two shard-row halves: the first half fires
mid-way through the previous layer's block loop and hides under compute.
The one-hot S tiles (fp8, 1.0 entries) and the gather index table are
shared by all 4 layers (same graph).

v2: the two d_h=256 layers (uf2/uf3) carry fp8 messages on the wire and
through the gather + one-hot matmuls (self-loop seed stays bf16 via a
separate bf16 copy of u); relu+residual+mean-accum fused into one
scalar_tensor_tensor.

v3: software-pipelined block loop.  Each block's work is split into
pass A (aggregate + PSUM evac into h, accumulating LN row sums) and
pass B (LN stats -> z -> W matmul -> u stores), with pass B emitted
DELTA blocks behind pass A so every engine queue always holds ready
work (the in-order sequencers otherwise serialize the ~14-op
cross-engine chain per block).  LN stats are batched over block pairs
and u/vt/out DMAs cover two blocks each (dma_start issue on the sync
sequencer costs ~0.8us).

kernel(**inputs) takes FULL numpy inputs, returns FULL [100000,128] out.
"""

import os
import numpy as np
import ml_dtypes

import concourse.bass as bass
import concourse.bacc as bacc
import concourse.mybir as mybir
import concourse.tile as tile
from concourse.bass_utils import run_bass_kernel_spmd

BF16 = ml_dtypes.bfloat16
FP8 = ml_dtypes.float8_e4m3

N = 100000
E = 300000
NCORES = 8
NLOC = 12500
NPAD = 12544          # 98 * 128
NT = 98
HALF = NPAD // 2      # 6272 = 49 * 128 (collective split point)
NBH = NT // 2         # 49 blocks per half
NG = NPAD * NCORES    # 100352
NGH = HALF * NCORES   # 50176 rows per uf half-tensor
CHUNK = HALF * 4      # 25088 rows (4 ranks' halves) per gather chunk
NCHUNK = 4            # A0 A1 B0 B1
D_IN, D_H, D_OUT = 128, 256, 128
EPS = 1e-5
CALLG = 8             # groups per gather call (8*128 = 1024 idxs, ring cap)
NQ = 4                # SWDGE queues
VT_AHEAD = 3          # self-term DMA prefetch distance (in PAIRS)
DELTA = 3             # pass-B lag behind pass A, in blocks
GAH = 4               # gather emission lookahead, in blocks

F32 = mybir.dt.float32
BF = mybir.dt.bfloat16
I16 = mybir.dt.int16
F8 = mybir.dt.float8e4

SKIP_AGG = bool(int(os.environ.get("KERNEL_SKIP_AGG", "0")))
SKIP_CC = bool(int(os.environ.get("KERNEL_SKIP_CC", "0")))


# ---------------------------------------------------------------- host prep

def _wrap16(idx):
    """[M] -> [128, M//16]: position i -> (i%16, i//16), replicated x8."""
    M = len(idx)
    w = np.zeros((128, M // 16), dtype=np.int16)
    t = idx.reshape(M // 16, 16).T
    for g in range(8):
        w[g * 16:(g + 1) * 16, :] = t
    return w


def _prep_graph(edge_index):
    """Cell (chunk, dst-block) schedule for one-hot-matmul aggregation.

    Chunks: A0/A1 = first shard-halves of ranks 0-3 / 4-7, B0/B1 = second
    halves, matching the split AllGather output tensors ufA/ufB.
    Returns (gidx[8], S[8], calls, block_groups, need_call, M, Gtot, dinv);
    calls = [(chunk, row_off, n_rows), ...] in emission order;
    block_groups[b] = [(call_idx, slot, G), ...];  schedule uniform across
    cores (group counts maxed over cores)."""
    src = np.asarray(edge_index[0], dtype=np.int64)
    dst = np.asarray(edge_index[1], dtype=np.int64)
    deg = np.bincount(dst, minlength=N).astype(np.float64) + 1.0
    dinv = (1.0 / np.sqrt(deg)).astype(np.float32)

    r_arr = dst // NLOC
    dloc = dst - r_arr * NLOC
    b_arr = dloc // 128
    drow = dloc - b_arr * 128
    srank = src // NLOC
    spad = src % NLOC                      # row within shard (pad ignored)
    half = (spad >= HALF).astype(np.int64)
    quad = srank // 4
    c_arr = half * 2 + quad
    crow = (srank % 4) * HALF + spad - half * HALF

    counts = np.bincount(
        (r_arr * NCHUNK + c_arr) * NT + b_arr,
        minlength=NCORES * NCHUNK * NT).reshape(NCORES, NCHUNK, NT)
    k = -(-counts.max(axis=0) // 128)        # [NCHUNK, NT] groups per cell
    ngroups = k.sum(axis=1)                  # per chunk
    base = np.zeros((NCHUNK, NT), np.int64)  # group base within chunk
    base[:, 1:] = np.cumsum(k, axis=1)[:, :-1]

    ncalls = [-(-int(g) // CALLG) for g in ngroups]
    merged = []                              # (chunk, local_call)
    for i in range(max(ncalls)):
        for c in range(NCHUNK):
            if i < ncalls[c]:
                merged.append((c, i))
    calls = []
    cidx = {}
    goff = {}                                # (c, local_call) -> global G base
    off = 0
    for ci, (c, i) in enumerate(merged):
        g0 = i * CALLG
        ng = min(CALLG, int(ngroups[c]) - g0)
        calls.append((c, off, ng * 128))
        cidx[(c, i)] = ci
        goff[(c, i)] = off // 128
        off += ng * 128
    M = off
    Gtot = M // 128

    # gmap[c, g_loc] -> global group id
    gmap = np.zeros((NCHUNK, max(1, int(ngroups.max()))), np.int64)
    callof = np.zeros_like(gmap)
    for c in range(NCHUNK):
        for g in range(int(ngroups[c])):
            i = g // CALLG
            gmap[c, g] = goff[(c, i)] + g % CALLG
            callof[c, g] = cidx[(c, i)]

    block_groups = [[] for _ in range(NT)]
    need_call = np.full(NT, -1, np.int64)
    for b in range(NT):
        for c in range(NCHUNK):
            for j in range(int(k[c, b])):
                g = int(base[c, b]) + j
                ci = int(callof[c, g])
                block_groups[b].append((ci, g % CALLG, int(gmap[c, g])))
                need_call[b] = max(need_call[b], ci)

    gidx, Ss = [], []
    for r in range(NCORES):
        m = r_arr == r
        ec, eb = c_arr[m], b_arr[m]
        ecrow, edrow = crow[m], drow[m]
        cell = ec * NT + eb
        order = np.argsort(cell, kind="stable")
        cell_s = cell[order]
        starts = np.searchsorted(cell_s, np.arange(NCHUNK * NT))
        tpos = np.arange(len(cell_s)) - starts[cell_s]
        g_loc = base.reshape(-1)[cell_s] + tpos // 128
        G = gmap[cell_s // NT, g_loc]
        row = G * 128 + tpos % 128
        g_rows = np.zeros(M, np.int16)
        g_rows[row] = ecrow[order]
        S3 = np.zeros((128, Gtot, 128), FP8)
        S3[tpos % 128, G, edrow[order]] = 1.0
        gidx.append(_wrap16(g_rows))
        Ss.append(np.ascontiguousarray(S3.reshape(128, Gtot * 128)))
    return gidx, Ss, calls, block_groups, need_call, M, Gtot, dinv


def _rep(v, p=128):
    return np.ascontiguousarray(
        np.broadcast_to(np.asarray(v, np.float32), (p, len(v))))


MP_BUFS = 6


def _check_liveness(calls, block_groups, need_call):
    """Every matmul must read a msg tile within the last MP_BUFS of its
    chunk's pool tag, else pool rotation clobbers it.  Emission follows
    the GAH-block lookahead used by layer_loop."""
    emitted = {c: [] for c in range(NCHUNK)}
    ci = 0
    for b in range(NT):
        while ci <= need_call[min(b + GAH, NT - 1)]:
            emitted[calls[ci][0]].append(ci)
            ci += 1
        for (cj, slot, G) in block_groups[b]:
            c = calls[cj][0]
            assert cj in emitted[c][-MP_BUFS:], (
                f"block {b} reads call {cj} beyond pool depth "
                f"{emitted[c][-MP_BUFS - 2:]}")


# ---------------------------------------------------------------- builder

def _build(M, Gtot, calls, block_groups, need_call,
           use_brow1, use_brow2, use_fg, use_fb):
    _check_liveness(calls, block_groups, need_call)
    nc = bacc.Bacc(None, target_bir_lowering=False, num_swdge_queues=NQ)

    def param(name, shape, dt, out=False):
        return nc.declare_dram_parameter(name, shape, dt, isOutput=out)

    u0_own = param("u0_own", [NPAD, D_IN], BF)
    gidx = param("gidx", [128, M // 16], I16)
    S_p = param("S", [128, Gtot * 128], F8)
    dinv_p = param("dinv", [128, NT], F32)
    ident_p = param("ident", [128, 128], BF)
    w0_p = param("w0", [128, D_H], BF)
    w1_p = param("w1", [2, 128, D_H], BF)
    w2_p = param("w2", [2, 128, D_H], BF)
    w3_p = param("w3", [2, 128, D_OUT], BF)
    b0_p = param("b0r", [128, D_H], F32)
    b1_p = param("b1r", [128, D_H], F32)
    b2_p = param("b2r", [128, D_H], F32)
    b3_p = param("b3r", [128, D_OUT], F32)
    brow1_p = param("brow1r", [128, D_H], F32) if use_brow1 else None
    brow2_p = param("brow2r", [128, D_H], F32) if use_brow2 else None
    fg_p = param("fgr", [128, D_OUT], F32) if use_fg else None
    fb_p = param("fbr", [128, D_OUT], F32) if use_fb else None
    out_p = param("out", [NLOC, D_OUT], F32, out=True)

    ul0 = nc.dram_tensor("ul0", [NPAD, D_IN], BF)
    ul23_bf = nc.dram_tensor("ul23_bf", [NPAD, D_H], BF)   # self-term source
    ul23_f8 = nc.dram_tensor("ul23_f8", [NPAD, D_H], F8)   # wire source
    ul4 = nc.dram_tensor("ul4", [NPAD, D_OUT], BF)
    uf0 = [nc.dram_tensor(f"uf0{h}", [NGH, D_IN], BF, addr_space="Shared")
           for h in "AB"]
    uf2 = [nc.dram_tensor(f"uf2{h}", [NGH, D_H], F8, addr_space="Shared")
           for h in "AB"]
    uf3 = [nc.dram_tensor(f"uf3{h}", [NGH, D_H], F8, addr_space="Shared")
           for h in "AB"]
    uf4 = [nc.dram_tensor(f"uf4{h}", [NGH, D_OUT], BF, addr_space="Shared")
           for h in "AB"]

    AX = mybir.AxisListType.X
    AF = mybir.ActivationFunctionType
    OP = mybir.AluOpType

    with tile.TileContext(nc) as tc:
        with (
            tc.tile_pool(name="const", bufs=1) as cp,
            tc.tile_pool(name="hbuf", bufs=1) as hp,
            tc.tile_pool(name="work", bufs=4) as wp,
            tc.tile_pool(name="sqp", bufs=2) as qp,
            tc.tile_pool(name="pair", bufs=4) as pp,
            tc.tile_pool(name="vtp", bufs=VT_AHEAD + 2) as vp,
            tc.tile_pool(name="small", bufs=6) as sp,
            tc.tile_pool(name="msg", bufs=MP_BUFS) as mp,
            tc.tile_pool(name="psT", bufs=2, space="PSUM") as pT,
            tc.tile_pool(name="psM", bufs=2, space="PSUM") as pM,
            tc.tile_pool(name="psA", bufs=4, space="PSUM") as pA,
        ):
            def cload(par, shape, dt):
                t = cp.tile(shape, dt, tag=par.name)
                nc.sync.dma_start(t[:], par[:])
                return t

            def allgather_half(ul, uf, h):
                if SKIP_CC:
                    return
                nc.gpsimd.collective_compute(
                    "AllGather", OP.bypass,
                    ins=[ul[h * HALF:(h + 1) * HALF, :].opt()],
                    outs=[uf[h][:].opt()],
                    replica_groups=[list(range(NCORES))],
                )

            # Layer-1 collectives first (input staged to ul0: collectives
            # can't read IO tensors); they overlap the constant loads below.
            nc.sync.dma_start(ul0[0:HALF, :], u0_own[0:HALF, :])
            allgather_half(ul0, uf0, 0)
            nc.sync.dma_start(ul0[HALF:NPAD, :], u0_own[HALF:NPAD, :])
            allgather_half(ul0, uf0, 1)

            gi = cload(gidx, [128, M // 16], I16)
            S_sb = cload(S_p, [128, Gtot * 128], F8)
            S_v = S_sb.rearrange("p (g d) -> p g d", d=128)
            dv = cload(dinv_p, [128, NT], F32)
            idn = cload(ident_p, [128, 128], BF)
            w0 = cload(w0_p, [128, D_H], BF)

            def wload(par, d):
                t = cp.tile([128, 2, d], BF, tag=par.name)
                nc.sync.dma_start(t[:], par.rearrange("k p d -> p k d"))
                return t

            w1 = wload(w1_p, D_H)
            w2 = wload(w2_p, D_H)
            w3 = wload(w3_p, D_OUT)
            b0 = cload(b0_p, [128, D_H], F32)
            b1 = cload(b1_p, [128, D_H], F32)
            b2 = cload(b2_p, [128, D_H], F32)
            b3 = cload(b3_p, [128, D_OUT], F32)
            brow1 = cload(brow1_p, [128, D_H], F32) if use_brow1 else None
            brow2 = cload(brow2_p, [128, D_H], F32) if use_brow2 else None
            fg = cload(fg_p, [128, D_OUT], F32) if use_fg else None
            fb = cload(fb_p, [128, D_OUT], F32) if use_fb else None

            h_sb = hp.tile([128, NT, D_H], BF)

            def r3(t, d):
                return t.rearrange("(n p) d -> p n d", p=128)

            def emit_gather(ci, uf, d, dt):
                """One gather call -> flat msg tile; returns [128,G,d] view."""
                (c, off, n) = calls[ci]
                t = mp.tile([128, CALLG * d], dt, tag=f"m{c}")
                tv = t.rearrange("p (g d) -> p g d", d=d)
                src = uf[c // 2][(c % 2) * CHUNK:(c % 2 + 1) * CHUNK, :]
                nc.gpsimd.dma_gather(
                    tv[:, : n // 128, :], src,
                    gi[:, off // 16:(off + n) // 16], n, n, d,
                    queue_num=ci % NQ,
                )
                return tv

            def agg_block(b, tiles, vt, d):
                """Self-term seed + one-hot matmuls for block b -> psum."""
                ps = pA.tile([128, d], F32, tag="agg")
                groups = [] if SKIP_AGG else block_groups[b]
                nc.tensor.matmul(ps[:], idn[:], vt[:],
                                 start=True, stop=not groups)
                for j, (ci, slot, G) in enumerate(groups):
                    nc.tensor.matmul(ps[:], S_v[:, G, :], tiles[ci][:, slot, :],
                                     start=False, stop=(j == len(groups) - 1))
                return ps

            def transpose_mm(z_bf, w, d_out, kchunks, evac):
                """z_bf [128, kchunks*128] -> mm = z^T @ w in PSUM.
                evac: 'S' or 'V' engine for the transpose evacuation."""
                mm = pM.tile([128, d_out], F32, tag="mm")
                zt_ps = pT.tile([128, kchunks, 128], BF, tag="zt_ps")
                for kk in range(kchunks):
                    nc.tensor.transpose(
                        zt_ps[:, kk, :], z_bf[:, kk * 128:(kk + 1) * 128],
                        idn[:])
                zt = wp.tile([128, kchunks, 128], BF, tag="zt")
                if evac == "S":
                    nc.scalar.activation(zt[:], zt_ps[:], AF.Copy)
                else:
                    nc.vector.tensor_scalar_mul(zt[:], zt_ps[:], 1.0)
                for kk in range(kchunks):
                    nc.tensor.matmul(mm[:], zt[:, kk, :],
                                     w[:, kk, :] if kchunks > 1 else w[:],
                                     start=(kk == 0), stop=(kk == kchunks - 1))
                return mm

            # ---------------- LN stats over a block pair ------------------
            def pair_stats(p, st):
                """Consumes st['sums'] [128,2]; fills negmu/s/negmu_s [128,2].
                Variance via Square+accum per block (bias = per-block
                negmu), then batched sqrt/recip/scale over the pair."""
                d = st["d"]
                negmu = sp.tile([128, 2], F32, tag="negmu")
                nc.vector.tensor_scalar_mul(negmu[:], st["sums"][:], -1.0 / d)
                ssq = sp.tile([128, 2], F32, tag="ssq")
                for k in (0, 1):
                    sq = qp.tile([128, d], BF, tag="sq")
                    nc.scalar.activation(sq[:], st["h"][k], AF.Square,
                                         bias=negmu[:, k:k + 1],
                                         accum_out=ssq[:, k:k + 1])
                varp = sp.tile([128, 2], F32, tag="varp")
                nc.vector.tensor_scalar(varp[:], ssq[:], 1.0 / d, EPS,
                                        OP.mult, OP.add)
                sd = sp.tile([128, 2], F32, tag="sd")
                nc.scalar.sqrt(sd[:], varp[:])
                rstd = sp.tile([128, 2], F32, tag="rstd")
                nc.vector.reciprocal(rstd[:], sd[:])
                st["negmu"] = negmu
                st["rstd"] = rstd
                return negmu, rstd

            def ln_scale_pair(p, st):
                """s = rstd*dinv, negmu_s = negmu*s for the pair."""
                s = sp.tile([128, 2], F32, tag="s")
                nc.vector.tensor_tensor(s[:], st["rstd"][:],
                                        dv[:, 2 * p:2 * p + 2], OP.mult)
                negmu_s = sp.tile([128, 2], F32, tag="negmu_s")
                nc.vector.tensor_tensor(negmu_s[:], st["negmu"][:], s[:],
                                        OP.mult)
                return s, negmu_s

            def store_u_pair(p, w, brow, ul_bf, zs):
                """z (bf16) per block -> mm -> u bf16+fp8, pair-batched DMA."""
                ubf = pp.tile([128, 2, D_H], BF, tag="ubf")
                u8 = pp.tile([128, 2, D_H], F8, tag="u8")
                for k in (0, 1):
                    mm = transpose_mm(zs[k], w, D_H, 2, evac="S")
                    if brow is not None:
                        b = 2 * p + k
                        nc.vector.scalar_tensor_tensor(
                            ubf[:, k, :], brow[:], dv[:, b:b + 1], mm[:],
                            OP.mult, OP.add)
                    else:
                        nc.vector.tensor_scalar_mul(ubf[:, k, :], mm[:], 1.0)
                    nc.vector.tensor_scalar_mul(u8[:, k, :], ubf[:, k, :],
                                                1.0)
                nc.sync.dma_start(r3(ul_bf, D_H)[:, 2 * p:2 * p + 2, :],
                                  ubf[:])
                nc.sync.dma_start(r3(ul23_f8, D_H)[:, 2 * p:2 * p + 2, :],
                                  u8[:])

            # ---------------- pipelined layer loop ------------------------
            def layer_loop(uf, d, dt, ul_self, passA, stage1, stage2,
                           mid_pair, mid_cc=None):
                """passA(b, ps): immediate PSUM evac (writes h/sums).
                stage1(p)/stage2(p): pass-B pair stages, stage1 lagging
                DELTA blocks behind pass A and stage2 one pair-tick
                behind stage1 (so stage2's PE work never waits on fresh
                stage1 results).  Stages are emitted oldest-first within
                an iteration to keep each engine queue head ready.
                mid_cc fires right after stage2(mid_pair)."""
                tiles = {}
                vts = {}
                ci = 0
                lag2 = DELTA + 2 if stage2 is not None else None

                def vt_dma(pv):
                    vt = vp.tile([128, 2, d], BF, tag="vt")
                    nc.sync.dma_start(vt[:],
                                      r3(ul_self, d)[:, 2 * pv:2 * pv + 2, :])
                    vts[pv] = vt

                def stage(fn, b, is_last):
                    if fn is not None and 0 <= b < NT and b % 2 == 1:
                        pr = b // 2
                        fn(pr)
                        if is_last and mid_cc is not None and pr == mid_pair:
                            mid_cc()

                for pv in range(min(VT_AHEAD, NBH)):
                    vt_dma(pv)
                for b in range(NT):
                    while ci <= need_call[min(b + GAH, NT - 1)]:
                        tiles[ci] = emit_gather(ci, uf, d, dt)
                        ci += 1
                    pv = b // 2 + VT_AHEAD
                    if b % 2 == 0 and pv < NBH:
                        vt_dma(pv)
                    if stage2 is not None:
                        stage(stage2, b - lag2, True)
                        stage(stage1, b - DELTA, False)
                    else:
                        stage(stage1, b - DELTA, True)
                    ps = agg_block(b, tiles, vts[b // 2][:, b % 2, :], d)
                    passA(b, ps)
                end = NT + (lag2 if stage2 is not None else DELTA)
                for b in range(NT, end):
                    if stage2 is not None:
                        stage(stage2, b - lag2, True)
                        stage(stage1, b - DELTA, False)
                    else:
                        stage(stage1, b - DELTA, True)

            # ================= Layer 1 ====================================
            l1_st = {}

            def l1_passA(b, ps):
                p = b // 2
                if b % 2 == 0:
                    l1_st[p] = {"d": D_H,
                                "tbf": pp.tile([128, 2, D_IN], BF,
                                               tag="tbf", name="tbf")}
                nc.scalar.activation(l1_st[p]["tbf"][:, b % 2, :], ps[:],
                                     AF.Copy)

            l1_zs = {}

            def l1_stage1(p):
                st = l1_st.pop(p)
                st["sums"] = sp.tile([128, 2], F32, tag="sums",
                                     name="sums")
                st["h"] = [h_sb[:, 2 * p, :], h_sb[:, 2 * p + 1, :]]
                for k in (0, 1):
                    b = 2 * p + k
                    mm = transpose_mm(st["tbf"][:, k, :], w0, D_H, 1,
                                      evac="V")
                    t2 = wp.tile([128, D_H], F32, tag="t2")
                    nc.vector.scalar_tensor_tensor(
                        t2[:], mm[:], dv[:, b:b + 1], b0[:], OP.mult, OP.add)
                    # relu with row-sum accum, on vector (op1 bypass)
                    nc.vector.scalar_tensor_tensor(
                        h_sb[:, b, :], t2[:], 0.0, t2[:], OP.max, OP.bypass,
                        accum_out=st["sums"][:, k:k + 1])
                pair_stats(p, st)
                s, negmu_s = ln_scale_pair(p, st)
                zs = []
                for k in (0, 1):
                    z = wp.tile([128, D_H], BF, tag="z")
                    nc.scalar.activation(z[:], st["h"][k], AF.Identity,
                                         bias=negmu_s[:, k:k + 1],
                                         scale=s[:, k:k + 1])
                    zs.append(z)
                l1_zs[p] = zs

            def l1_stage2(p):
                store_u_pair(p, w1, brow1, ul23_bf, l1_zs.pop(p))

            layer_loop(uf0, D_IN, BF, u0_own, l1_passA, l1_stage1,
                       l1_stage2, mid_pair=NBH // 2,
                       mid_cc=lambda: allgather_half(ul23_f8, uf2, 0))
            allgather_half(ul23_f8, uf2, 1)

            # ================= Layers 2, 3 ================================
            def mk_mid(bias, st_map):
                def passA(b, ps):
                    p = b // 2
                    if b % 2 == 0:
                        st_map[p] = {"d": D_H,
                                     "sums": sp.tile([128, 2], F32,
                                                     tag="sums",
                                                     name="sums"),
                                     "h": [h_sb[:, 2 * p, :],
                                           h_sb[:, 2 * p + 1, :]]}
                    st = st_map[p]
                    t4 = wp.tile([128, D_H], F32, tag="t4")
                    nc.vector.scalar_tensor_tensor(
                        t4[:], ps[:], dv[:, b:b + 1], bias[:],
                        OP.mult, OP.add)
                    # h = relu(t4) + h_old, accumulating LN row sums
                    nc.vector.scalar_tensor_tensor(
                        h_sb[:, b, :], t4[:], 0.0, h_sb[:, b, :],
                        OP.max, OP.add, accum_out=st["sums"][:, b % 2:
                                                             b % 2 + 1])
                return passA

            def mk_mid_stage1(st_map, zs_map):
                def stage1(p):
                    st = st_map.pop(p)
                    pair_stats(p, st)
                    s, negmu_s = ln_scale_pair(p, st)
                    zs = []
                    for k in (0, 1):
                        z = wp.tile([128, D_H], BF, tag="z")
                        nc.scalar.activation(z[:], st["h"][k], AF.Identity,
                                             bias=negmu_s[:, k:k + 1],
                                             scale=s[:, k:k + 1])
                        zs.append(z)
                    zs_map[p] = zs
                return stage1

            def mk_mid_stage2(zs_map, w, brow):
                def stage2(p):
                    store_u_pair(p, w, brow, ul23_bf, zs_map.pop(p))
                return stage2

            l2_st, l2_zs = {}, {}
            layer_loop(uf2, D_H, F8, ul23_bf, mk_mid(b1, l2_st),
                       mk_mid_stage1(l2_st, l2_zs),
                       mk_mid_stage2(l2_zs, w2, brow2),
                       mid_pair=NBH // 2,
                       mid_cc=lambda: allgather_half(ul23_f8, uf3, 0))
            allgather_half(ul23_f8, uf3, 1)

            l3_st, l3_zs = {}, {}

            def l3_stage1(p):
                # u-compute for layer 4: (h * dinv) @ W3  (no LN)
                st = l3_st.pop(p)
                zs = []
                for k in (0, 1):
                    b = 2 * p + k
                    z = wp.tile([128, D_H], BF, tag="z")
                    nc.vector.tensor_scalar_mul(z[:], st["h"][k],
                                                dv[:, b:b + 1])
                    zs.append(z)
                l3_zs[p] = zs

            def l3_stage2(p):
                zs = l3_zs.pop(p)
                ubf = pp.tile([128, 2, D_OUT], BF, tag="ub4")
                for k in (0, 1):
                    mm = transpose_mm(zs[k], w3, D_OUT, 2, evac="S")
                    nc.scalar.activation(ubf[:, k, :], mm[:], AF.Copy)
                nc.sync.dma_start(r3(ul4, D_OUT)[:, 2 * p:2 * p + 2, :],
                                  ubf[:])

            layer_loop(uf3, D_H, F8, ul23_bf, mk_mid(b2, l3_st), l3_stage1,
                       l3_stage2, mid_pair=NBH // 2,
                       mid_cc=lambda: allgather_half(ul4, uf4, 0))
            allgather_half(ul4, uf4, 1)

            # ================= Layer 4 ====================================
            l4_st = {}

            def l4_passA(b, ps):
                p = b // 2
                if b % 2 == 0:
                    l4_st[p] = {"d": D_OUT,
                                "sums": sp.tile([128, 2], F32, tag="sums",
                                                name="sums"),
                                "y2": pp.tile([128, 2, D_OUT], F32,
                                              tag="y2", name="y2")}
                st = l4_st[p]
                nc.vector.scalar_tensor_tensor(
                    st["y2"][:, b % 2, :], ps[:], dv[:, b:b + 1], b3[:],
                    OP.mult, OP.add, accum_out=st["sums"][:, b % 2:b % 2 + 1])

            def l4_workP(p):
                st = l4_st.pop(p)
                st["h"] = [st["y2"][:, 0, :], st["y2"][:, 1, :]]
                pair_stats(p, st)
                zo = pp.tile([128, 2, D_OUT], F32, tag="zo")
                for k in (0, 1):
                    nc.vector.tensor_scalar(
                        zo[:, k, :], st["y2"][:, k, :],
                        st["negmu"][:, k:k + 1], st["rstd"][:, k:k + 1],
                        OP.add, OP.mult)
                    if fg is not None:
                        nc.vector.tensor_tensor(zo[:, k, :], zo[:, k, :],
                                                fg[:], OP.mult)
                    if fb is not None:
                        nc.vector.tensor_tensor(zo[:, k, :], zo[:, k, :],
                                                fb[:], OP.add)
                lo = 2 * p * 128
                if lo + 256 <= NLOC:
                    nc.sync.dma_start(
                        out_p[lo:lo + 256, :].rearrange(
                            "(n p) d -> p n d", p=128), zo[:])
                else:
                    for k in (0, 1):
                        l2_ = lo + k * 128
                        nrow = min(128, NLOC - l2_)
                        if nrow > 0:
                            nc.sync.dma_start(out_p[l2_:l2_ + nrow, :],
                                              zo[0:nrow, k, :])

            layer_loop(uf4, D_OUT, BF, ul4, l4_passA, l4_workP, None,
                       mid_pair=10**9)

    nc.compile()
    return nc


_CACHE = {}


def kernel(x, edge_index, W0, b0, W1, b1, W2, b2, W3, b3,
           ln0_g, ln0_b, ln1_g, ln1_b, fln_g, fln_b):
    x = np.asarray(x, np.float32)
    edge_index = np.asarray(edge_index)
    (gidx, Ss, calls, block_groups, need_call, M, Gtot,
     dinv) = _prep_graph(edge_index)

    W1f = np.asarray(ln0_g, np.float32)[:, None] * np.asarray(W1, np.float32)
    W2f = np.asarray(ln1_g, np.float32)[:, None] * np.asarray(W2, np.float32)
    brow1 = np.asarray(ln0_b, np.float32) @ np.asarray(W1, np.float32)
    brow2 = np.asarray(ln1_b, np.float32) @ np.asarray(W2, np.float32)
    use_brow1 = bool(np.any(brow1 != 0))
    use_brow2 = bool(np.any(brow2 != 0))
    use_fg = bool(np.any(np.asarray(fln_g) != 1))
    use_fb = bool(np.any(np.asarray(fln_b) != 0))

    key = (M, Gtot, tuple(calls), tuple(need_call),
           tuple(tuple(g) for g in block_groups),
           use_brow1, use_brow2, use_fg, use_fb)
    if key not in _CACHE:
        _CACHE[key] = _build(M, Gtot, calls, block_groups, need_call,
                             use_brow1, use_brow2, use_fg, use_fb)
    nc = _CACHE[key]

    u0 = dinv[:, None].astype(np.float32) * x
    u0p = np.zeros((NCORES, NPAD, D_IN), BF16)
    for r in range(NCORES):
        u0p[r, :NLOC] = u0[r * NLOC:(r + 1) * NLOC]
    dinv_pad = np.zeros((NCORES, NPAD), np.float32)
    for r in range(NCORES):
        dinv_pad[r, :NLOC] = dinv[r * NLOC:(r + 1) * NLOC]

    def chunk2(Wf):
        return np.stack([Wf[0:128], Wf[128:256]]).astype(BF16)

    common = {
        "ident": np.eye(128, dtype=BF16),
        "w0": np.asarray(W0, np.float32).astype(BF16),
        "w1": chunk2(W1f), "w2": chunk2(W2f),
        "w3": chunk2(np.asarray(W3, np.float32)),
        "b0r": _rep(b0), "b1r": _rep(b1), "b2r": _rep(b2), "b3r": _rep(b3),
    }
    if use_brow1:
        common["brow1r"] = _rep(brow1)
    if use_brow2:
        common["brow2r"] = _rep(brow2)
    if use_fg:
        common["fgr"] = _rep(fln_g)
    if use_fb:
        common["fbr"] = _rep(fln_b)

    in_maps = []
    for r in range(NCORES):
        m = dict(common)
        m["u0_own"] = u0p[r]
        m["gidx"] = gidx[r]
        m["S"] = Ss[r]
        m["dinv"] = np.ascontiguousarray(dinv_pad[r].reshape(NT, 128).T)
        in_maps.append(m)

    res = run_bass_kernel_spmd(nc, in_maps, core_ids=list(range(NCORES)))
    out = np.concatenate([res.results[r]["out"] for r in range(NCORES)],
                         axis=0)
    return out.astype(np.float32)


# revision 18
# speedup vs baseline: 1.0334x; 1.0127x over previous
"""4-layer GCN (ArithmeticCircuitGNN) on 8 Trainium2 NeuronCores.

Node-parallel, aggregation on the TENSOR engine: 12544-padded shard/core,
LN affine folded into weights on host.  Per GCN layer:
  AllGather(u) -> dma_gather(u[src]) in (chunk, dst-block) cells ->
  one-hot matmul S^T @ msg accumulated in PSUM per dst block, seeded with
  the self-loop term via an identity matmul -> evac (*dinv_dst, +bias,
  relu, +residual).  No DRAM scatter-add, no V planes.
Each AllGather is split in two shard-row halves: the first half fires
mid-way through the previous layer's block loop and hides under compute.
The one-hot S tiles (fp8, 1.0 entries) and the gather index table are
shared by all 4 layers (same graph).

v2: the two d_h=256 layers (uf2/uf3) carry fp8 messages on the wire and
through the gather + one-hot matmuls (self-loop seed stays bf16 via a
separate bf16 copy of u); relu+residual+mean-accum fused into one
scalar_tensor_tensor.

v3: software-pipelined block loop.  Each block's work is split into
pass A (aggregate + PSUM evac into h, accumulating LN row sums) and
pass B (LN stats -> z -> W matmul -> u stores), with pass B emitted
DELTA blocks behind pass A so every engine queue always holds ready
work (the in-order sequencers otherwise serialize the ~14-op
cross-engine chain per block).  LN stats are batched over block pairs
and u/vt/out DMAs cover two blocks each (dma_start issue on the sync
sequencer costs ~0.8us).

kernel(**inputs) takes FULL numpy inputs, returns FULL [100000,128] out.
"""

import os
import numpy as np
import ml_dtypes

import concourse.bass as bass
import concourse.bacc as bacc
import concourse.mybir as mybir
import concourse.tile as tile
from concourse.bass_utils import run_bass_kernel_spmd

BF16 = ml_dtypes.bfloat16
FP8 = ml_dtypes.float8_e4m3

N = 100000
E = 300000
NCORES = 8
NLOC = 12500
NPAD = 12544          # 98 * 128
NT = 98
HALF = NPAD // 2      # 6272 = 49 * 128 (collective split point)
NBH = NT // 2         # 49 blocks per half
NG = NPAD * NCORES    # 100352
NGH = HALF * NCORES   # 50176 rows per uf half-tensor
CHUNK = HALF * 4      # 25088 rows (4 ranks' halves) per gather chunk
NCHUNK = 4            # A0 A1 B0 B1
D_IN, D_H, D_OUT = 128, 256, 128
EPS = 1e-5
CALLG = 8             # groups per gather call (8*128 = 1024 idxs, ring cap)
NQ = 4                # SWDGE queues
VT_AHEAD = 3          # self-term DMA prefetch distance (in PAIRS)
DELTA = 3             # pass-B lag behind pass A, in blocks
GAH = 4               # gather emission lookahead, in blocks

F32 = mybir.dt.float32
BF = mybir.dt.bfloat16
I16 = mybir.dt.int16
F8 = mybir.dt.float8e4

SKIP_AGG = bool(int(os.environ.get("KERNEL_SKIP_AGG", "0")))
SKIP_CC = bool(int(os.environ.get("KERNEL_SKIP_CC", "0")))


# ---------------------------------------------------------------- host prep

def _wrap16(idx):
    """[M] -> [128, M//16]: position i -> (i%16, i//16), replicated x8."""
    M = len(idx)
    w = np.zeros((128, M // 16), dtype=np.int16)
    t = idx.reshape(M // 16, 16).T
    for g in range(8):
        w[g * 16:(g + 1) * 16, :] = t
    return w


def _prep_graph(edge_index):
    """Cell (chunk, dst-block) schedule for one-hot-matmul aggregation.

    Chunks: A0/A1 = first shard-halves of ranks 0-3 / 4-7, B0/B1 = second
    halves, matching the split AllGather output tensors ufA/ufB.
    Returns (gidx[8], S[8], calls, block_groups, need_call, M, Gtot, dinv);
    calls = [(chunk, row_off, n_rows), ...] in emission order;
    block_groups[b] = [(call_idx, slot, G), ...];  schedule uniform across
    cores (group counts maxed over cores)."""
    src = np.asarray(edge_index[0], dtype=np.int64)
    dst = np.asarray(edge_index[1], dtype=np.int64)
    deg = np.bincount(dst, minlength=N).astype(np.float64) + 1.0
    dinv = (1.0 / np.sqrt(deg)).astype(np.float32)

    r_arr = dst // NLOC
    dloc = dst - r_arr * NLOC
    b_arr = dloc // 128
    drow = dloc - b_arr * 128
    srank = src // NLOC
    spad = src % NLOC                      # row within shard (pad ignored)
    half = (spad >= HALF).astype(np.int64)
    quad = srank // 4
    c_arr = half * 2 + quad
    crow = (srank % 4) * HALF + spad - half * HALF

    counts = np.bincount(
        (r_arr * NCHUNK + c_arr) * NT + b_arr,
        minlength=NCORES * NCHUNK * NT).reshape(NCORES, NCHUNK, NT)
    k = -(-counts.max(axis=0) // 128)        # [NCHUNK, NT] groups per cell
    ngroups = k.sum(axis=1)                  # per chunk
    base = np.zeros((NCHUNK, NT), np.int64)  # group base within chunk
    base[:, 1:] = np.cumsum(k, axis=1)[:, :-1]

    ncalls = [-(-int(g) // CALLG) for g in ngroups]
    merged = []                              # (chunk, local_call)
    for i in range(max(ncalls)):
        for c in range(NCHUNK):
            if i < ncalls[c]:
                merged.append((c, i))
    calls = []
    cidx = {}
    goff = {}                                # (c, local_call) -> global G base
    off = 0
    for ci, (c, i) in enumerate(merged):
        g0 = i * CALLG
        ng = min(CALLG, int(ngroups[c]) - g0)
        calls.append((c, off, ng * 128))
        cidx[(c, i)] = ci
        goff[(c, i)] = off // 128
        off += ng * 128
    M = off
    Gtot = M // 128

    # gmap[c, g_loc] -> global group id
    gmap = np.zeros((NCHUNK, max(1, int(ngroups.max()))), np.int64)
    callof = np.zeros_like(gmap)
    for c in range(NCHUNK):
        for g in range(int(ngroups[c])):
            i = g // CALLG
            gmap[c, g] = goff[(c, i)] + g % CALLG
            callof[c, g] = cidx[(c, i)]

    block_groups = [[] for _ in range(NT)]
    need_call = np.full(NT, -1, np.int64)
    for b in range(NT):
        for c in range(NCHUNK):
            for j in range(int(k[c, b])):
                g = int(base[c, b]) + j
                ci = int(callof[c, g])
                block_groups[b].append((ci, g % CALLG, int(gmap[c, g])))
                need_call[b] = max(need_call[b], ci)

    gidx, Ss = [], []
    for r in range(NCORES):
        m = r_arr == r
        ec, eb = c_arr[m], b_arr[m]
        ecrow, edrow = crow[m], drow[m]
        cell = ec * NT + eb
        order = np.argsort(cell, kind="stable")
        cell_s = cell[order]
        starts = np.searchsorted(cell_s, np.arange(NCHUNK * NT))
        tpos = np.arange(len(cell_s)) - starts[cell_s]
        g_loc = base.reshape(-1)[cell_s] + tpos // 128
        G = gmap[cell_s // NT, g_loc]
        row = G * 128 + tpos % 128
        g_rows = np.zeros(M, np.int16)
        g_rows[row] = ecrow[order]
        S3 = np.zeros((128, Gtot, 128), FP8)
        S3[tpos % 128, G, edrow[order]] = 1.0
        gidx.append(_wrap16(g_rows))
        Ss.append(np.ascontiguousarray(S3.reshape(128, Gtot * 128)))
    return gidx, Ss, calls, block_groups, need_call, M, Gtot, dinv


def _rep(v, p=128):
    return np.ascontiguousarray(
        np.broadcast_to(np.asarray(v, np.float32), (p, len(v))))


MP_BUFS = 6


def _check_liveness(calls, block_groups, need_call):
    """Every matmul must read a msg tile within the last MP_BUFS of its
    chunk's pool tag, else pool rotation clobbers it.  Emission follows
    the GAH-block lookahead used by layer_loop."""
    emitted = {c: [] for c in range(NCHUNK)}
    ci = 0
    for b in range(NT):
        while ci <= need_call[min(b + GAH, NT - 1)]:
            emitted[calls[ci][0]].append(ci)
            ci += 1
        for (cj, slot, G) in block_groups[b]:
            c = calls[cj][0]
            assert cj in emitted[c][-MP_BUFS:], (
                f"block {b} reads call {cj} beyond pool depth "
                f"{emitted[c][-MP_BUFS - 2:]}")


# ---------------------------------------------------------------- builder

def _build(M, Gtot, calls, block_groups, need_call,
           use_brow1, use_brow2, use_fg, use_fb):
    _check_liveness(calls, block_groups, need_call)
    nc = bacc.Bacc(None, target_bir_lowering=False, num_swdge_queues=NQ)

    def param(name, shape, dt, out=False):
        return nc.declare_dram_parameter(name, shape, dt, isOutput=out)

    u0_own = param("u0_own", [NPAD, D_IN], BF)
    gidx = param("gidx", [128, M // 16], I16)
    S_p = param("S", [128, Gtot * 128], F8)
    dinv_p = param("dinv", [128, NT], F32)
    ident_p = param("ident", [128, 128], BF)
    w0_p = param("w0", [128, D_H], BF)
    w1_p = param("w1", [2, 128, D_H], BF)
    w2_p = param("w2", [2, 128, D_H], BF)
    w3_p = param("w3", [2, 128, D_OUT], BF)
    b0_p = param("b0r", [128, D_H], F32)
    b1_p = param("b1r", [128, D_H], F32)
    b2_p = param("b2r", [128, D_H], F32)
    b3_p = param("b3r", [128, D_OUT], F32)
    brow1_p = param("brow1r", [128, D_H], F32) if use_brow1 else None
    brow2_p = param("brow2r", [128, D_H], F32) if use_brow2 else None
    fg_p = param("fgr", [128, D_OUT], F32) if use_fg else None
    fb_p = param("fbr", [128, D_OUT], F32) if use_fb else None
    out_p = param("out", [NLOC, D_OUT], F32, out=True)

    ul0 = nc.dram_tensor("ul0", [NPAD, D_IN], BF)
    ul23_bf = nc.dram_tensor("ul23_bf", [NPAD, D_H], BF)   # self-term source
    ul23_f8 = nc.dram_tensor("ul23_f8", [NPAD, D_H], F8)   # wire source
    ul4 = nc.dram_tensor("ul4", [NPAD, D_OUT], BF)
    uf0 = [nc.dram_tensor(f"uf0{h}", [NGH, D_IN], BF, addr_space="Shared")
           for h in "AB"]
    uf2 = [nc.dram_tensor(f"uf2{h}", [NGH, D_H], F8, addr_space="Shared")
           for h in "AB"]
    uf3 = [nc.dram_tensor(f"uf3{h}", [NGH, D_H], F8, addr_space="Shared")
           for h in "AB"]
    uf4 = [nc.dram_tensor(f"uf4{h}", [NGH, D_OUT], BF, addr_space="Shared")
           for h in "AB"]

    AX = mybir.AxisListType.X
    AF = mybir.ActivationFunctionType
    OP = mybir.AluOpType

    with tile.TileContext(nc) as tc:
        with (
            tc.tile_pool(name="const", bufs=1) as cp,
            tc.tile_pool(name="hbuf", bufs=1) as hp,
            tc.tile_pool(name="work", bufs=4) as wp,
            tc.tile_pool(name="sqp", bufs=2) as qp,
            tc.tile_pool(name="pair", bufs=4) as pp,
            tc.tile_pool(name="vtp", bufs=VT_AHEAD + 2) as vp,
            tc.tile_pool(name="small", bufs=6) as sp,
            tc.tile_pool(name="msg", bufs=MP_BUFS) as mp,
            tc.tile_pool(name="psT", bufs=2, space="PSUM") as pT,
            tc.tile_pool(name="psM", bufs=2, space="PSUM") as pM,
            tc.tile_pool(name="psA", bufs=4, space="PSUM") as pA,
        ):
            def cload(par, shape, dt):
                t = cp.tile(shape, dt, tag=par.name)
                nc.sync.dma_start(t[:], par[:])
                return t

            def allgather_half(ul, uf, h):
                if SKIP_CC:
                    return
                nc.gpsimd.collective_compute(
                    "AllGather", OP.bypass,
                    ins=[ul[h * HALF:(h + 1) * HALF, :].opt()],
                    outs=[uf[h][:].opt()],
                    replica_groups=[list(range(NCORES))],
                )

            QTR = HALF // 2

            def allgather_b_quarter(ul, uf, q):
                """Quarter q of the second shard-half; output rows are
                rank-strided slices of uf[1] (rank r rows at r*HALF+q*QTR)."""
                if SKIP_CC:
                    return
                d = ul.shape[1]
                ufv = uf[1].rearrange("(r q x) d -> r q (x d)", r=NCORES, q=2)
                nc.gpsimd.collective_compute(
                    "AllGather", OP.bypass,
                    ins=[ul[HALF + q * QTR:HALF + (q + 1) * QTR, :].opt()],
                    outs=[ufv[:, q, :].opt()],
                    replica_groups=[list(range(NCORES))],
                )

            # Layer-1 collectives first (input staged to ul0: collectives
            # can't read IO tensors); they overlap the constant loads below.
            nc.sync.dma_start(ul0[0:HALF, :], u0_own[0:HALF, :])
            allgather_half(ul0, uf0, 0)
            nc.sync.dma_start(ul0[HALF:NPAD, :], u0_own[HALF:NPAD, :])
            allgather_half(ul0, uf0, 1)

            gi = cload(gidx, [128, M // 16], I16)
            S_sb = cload(S_p, [128, Gtot * 128], F8)
            S_v = S_sb.rearrange("p (g d) -> p g d", d=128)
            dv = cload(dinv_p, [128, NT], F32)
            idn = cload(ident_p, [128, 128], BF)
            w0 = cload(w0_p, [128, D_H], BF)

            def wload(par, d):
                t = cp.tile([128, 2, d], BF, tag=par.name)
                nc.sync.dma_start(t[:], par.rearrange("k p d -> p k d"))
                return t

            w1 = wload(w1_p, D_H)
            w2 = wload(w2_p, D_H)
            w3 = wload(w3_p, D_OUT)
            b0 = cload(b0_p, [128, D_H], F32)
            b1 = cload(b1_p, [128, D_H], F32)
            b2 = cload(b2_p, [128, D_H], F32)
            b3 = cload(b3_p, [128, D_OUT], F32)
            brow1 = cload(brow1_p, [128, D_H], F32) if use_brow1 else None
            brow2 = cload(brow2_p, [128, D_H], F32) if use_brow2 else None
            fg = cload(fg_p, [128, D_OUT], F32) if use_fg else None
            fb = cload(fb_p, [128, D_OUT], F32) if use_fb else None

            h_sb = hp.tile([128, NT, D_H], BF)

            def r3(t, d):
                return t.rearrange("(n p) d -> p n d", p=128)

            def emit_gather(ci, uf, d, dt):
                """One gather call -> flat msg tile; returns [128,G,d] view."""
                (c, off, n) = calls[ci]
                t = mp.tile([128, CALLG * d], dt, tag=f"m{c}")
                tv = t.rearrange("p (g d) -> p g d", d=d)
                src = uf[c // 2][(c % 2) * CHUNK:(c % 2 + 1) * CHUNK, :]
                nc.gpsimd.dma_gather(
                    tv[:, : n // 128, :], src,
                    gi[:, off // 16:(off + n) // 16], n, n, d,
                    queue_num=ci % NQ,
                )
                return tv

            def agg_block(b, tiles, vt, d):
                """Self-term seed + one-hot matmuls for block b -> psum."""
                ps = pA.tile([128, d], F32, tag="agg")
                groups = [] if SKIP_AGG else block_groups[b]
                nc.tensor.matmul(ps[:], idn[:], vt[:],
                                 start=True, stop=not groups)
                for j, (ci, slot, G) in enumerate(groups):
                    nc.tensor.matmul(ps[:], S_v[:, G, :], tiles[ci][:, slot, :],
                                     start=False, stop=(j == len(groups) - 1))
                return ps

            def transpose_mm(z_bf, w, d_out, kchunks, evac):
                """z_bf [128, kchunks*128] -> mm = z^T @ w in PSUM.
                evac: 'S' or 'V' engine for the transpose evacuation."""
                mm = pM.tile([128, d_out], F32, tag="mm")
                zt_ps = pT.tile([128, kchunks, 128], BF, tag="zt_ps")
                for kk in range(kchunks):
                    nc.tensor.transpose(
                        zt_ps[:, kk, :], z_bf[:, kk * 128:(kk + 1) * 128],
                        idn[:])
                zt = wp.tile([128, kchunks, 128], BF, tag="zt")
                if evac == "S":
                    nc.scalar.activation(zt[:], zt_ps[:], AF.Copy)
                else:
                    nc.vector.tensor_scalar_mul(zt[:], zt_ps[:], 1.0)
                for kk in range(kchunks):
                    nc.tensor.matmul(mm[:], zt[:, kk, :],
                                     w[:, kk, :] if kchunks > 1 else w[:],
                                     start=(kk == 0), stop=(kk == kchunks - 1))
                return mm

            # ---------------- LN stats over a block pair ------------------
            def pair_stats(p, st):
                """Consumes st['sums'] [128,2]; fills negmu/s/negmu_s [128,2].
                Variance via Square+accum per block (bias = per-block
                negmu), then batched sqrt/recip/scale over the pair."""
                d = st["d"]
                negmu = sp.tile([128, 2], F32, tag="negmu")
                nc.vector.tensor_scalar_mul(negmu[:], st["sums"][:], -1.0 / d)
                ssq = sp.tile([128, 2], F32, tag="ssq")
                for k in (0, 1):
                    sq = qp.tile([128, d], BF, tag="sq")
                    nc.scalar.activation(sq[:], st["h"][k], AF.Square,
                                         bias=negmu[:, k:k + 1],
                                         accum_out=ssq[:, k:k + 1])
                varp = sp.tile([128, 2], F32, tag="varp")
                nc.vector.tensor_scalar(varp[:], ssq[:], 1.0 / d, EPS,
                                        OP.mult, OP.add)
                sd = sp.tile([128, 2], F32, tag="sd")
                nc.scalar.sqrt(sd[:], varp[:])
                rstd = sp.tile([128, 2], F32, tag="rstd")
                nc.vector.reciprocal(rstd[:], sd[:])
                st["negmu"] = negmu
                st["rstd"] = rstd
                return negmu, rstd

            def ln_scale_pair(p, st):
                """s = rstd*dinv, negmu_s = negmu*s for the pair."""
                s = sp.tile([128, 2], F32, tag="s")
                nc.vector.tensor_tensor(s[:], st["rstd"][:],
                                        dv[:, 2 * p:2 * p + 2], OP.mult)
                negmu_s = sp.tile([128, 2], F32, tag="negmu_s")
                nc.vector.tensor_tensor(negmu_s[:], st["negmu"][:], s[:],
                                        OP.mult)
                return s, negmu_s

            def store_u_pair(p, w, brow, ul_bf, zs):
                """z (bf16) per block -> mm -> u bf16+fp8, pair-batched DMA."""
                ubf = pp.tile([128, 2, D_H], BF, tag="ubf")
                u8 = pp.tile([128, 2, D_H], F8, tag="u8")
                for k in (0, 1):
                    mm = transpose_mm(zs[k], w, D_H, 2, evac="S")
                    if brow is not None:
                        b = 2 * p + k
                        nc.vector.scalar_tensor_tensor(
                            ubf[:, k, :], brow[:], dv[:, b:b + 1], mm[:],
                            OP.mult, OP.add)
                    else:
                        nc.vector.tensor_scalar_mul(ubf[:, k, :], mm[:], 1.0)
                    nc.vector.tensor_scalar_mul(u8[:, k, :], ubf[:, k, :],
                                                1.0)
                nc.sync.dma_start(r3(ul_bf, D_H)[:, 2 * p:2 * p + 2, :],
                                  ubf[:])
                nc.sync.dma_start(r3(ul23_f8, D_H)[:, 2 * p:2 * p + 2, :],
                                  u8[:])

            # ---------------- pipelined layer loop ------------------------
            def layer_loop(uf, d, dt, ul_self, passA, stage1, stage2,
                           cc_map=None):
                """passA(b, ps): immediate PSUM evac (writes h/sums).
                stage1(p)/stage2(p): pass-B pair stages, stage1 lagging
                DELTA blocks behind pass A and stage2 one pair-tick
                behind stage1 (so stage2's PE work never waits on fresh
                stage1 results).  Stages are emitted oldest-first within
                an iteration to keep each engine queue head ready.
                mid_cc fires right after stage2(mid_pair)."""
                tiles = {}
                vts = {}
                ci = 0
                lag2 = DELTA + 2 if stage2 is not None else None
                cc_map = cc_map or {}

                def vt_dma(pv):
                    vt = vp.tile([128, 2, d], BF, tag="vt")
                    nc.sync.dma_start(vt[:],
                                      r3(ul_self, d)[:, 2 * pv:2 * pv + 2, :])
                    vts[pv] = vt

                def stage(fn, b, is_last):
                    if fn is not None and 0 <= b < NT and b % 2 == 1:
                        pr = b // 2
                        fn(pr)
                        if is_last and pr in cc_map:
                            cc_map[pr]()

                for pv in range(min(VT_AHEAD, NBH)):
                    vt_dma(pv)
                for b in range(NT):
                    while ci <= need_call[min(b + GAH, NT - 1)]:
                        tiles[ci] = emit_gather(ci, uf, d, dt)
                        ci += 1
                    pv = b // 2 + VT_AHEAD
                    if b % 2 == 0 and pv < NBH:
                        vt_dma(pv)
                    if stage2 is not None:
                        stage(stage2, b - lag2, True)
                        stage(stage1, b - DELTA, False)
                    else:
                        stage(stage1, b - DELTA, True)
                    ps = agg_block(b, tiles, vts[b // 2][:, b % 2, :], d)
                    passA(b, ps)
                end = NT + (lag2 if stage2 is not None else DELTA)
                for b in range(NT, end):
                    if stage2 is not None:
                        stage(stage2, b - lag2, True)
                        stage(stage1, b - DELTA, False)
                    else:
                        stage(stage1, b - DELTA, True)

            # ================= Layer 1 ====================================
            l1_st = {}

            def l1_passA(b, ps):
                p = b // 2
                if b % 2 == 0:
                    l1_st[p] = {"d": D_H,
                                "tbf": pp.tile([128, 2, D_IN], BF,
                                               tag="tbf", name="tbf")}
                nc.scalar.activation(l1_st[p]["tbf"][:, b % 2, :], ps[:],
                                     AF.Copy)

            l1_zs = {}

            def l1_stage1(p):
                st = l1_st.pop(p)
                st["sums"] = sp.tile([128, 2], F32, tag="sums",
                                     name="sums")
                st["h"] = [h_sb[:, 2 * p, :], h_sb[:, 2 * p + 1, :]]
                for k in (0, 1):
                    b = 2 * p + k
                    mm = transpose_mm(st["tbf"][:, k, :], w0, D_H, 1,
                                      evac="V")
                    t2 = wp.tile([128, D_H], F32, tag="t2")
                    nc.vector.scalar_tensor_tensor(
                        t2[:], mm[:], dv[:, b:b + 1], b0[:], OP.mult, OP.add)
                    # relu with row-sum accum, on vector (op1 bypass)
                    nc.vector.scalar_tensor_tensor(
                        h_sb[:, b, :], t2[:], 0.0, t2[:], OP.max, OP.bypass,
                        accum_out=st["sums"][:, k:k + 1])
                pair_stats(p, st)
                s, negmu_s = ln_scale_pair(p, st)
                zs = []
                for k in (0, 1):
                    z = wp.tile([128, D_H], BF, tag="z")
                    nc.scalar.activation(z[:], st["h"][k], AF.Identity,
                                         bias=negmu_s[:, k:k + 1],
                                         scale=s[:, k:k + 1])
                    zs.append(z)
                l1_zs[p] = zs

            def l1_stage2(p):
                store_u_pair(p, w1, brow1, ul23_bf, l1_zs.pop(p))

            layer_loop(uf0, D_IN, BF, u0_own, l1_passA,
                       lambda p: (l1_stage1(p), l1_stage2(p)), None,
                       cc_map={
                           NBH // 2: lambda: allgather_half(ul23_f8, uf2, 0)})
            allgather_half(ul23_f8, uf2, 1)

            # ================= Layers 2, 3 ================================
            def mk_mid(bias, st_map):
                def passA(b, ps):
                    p = b // 2
                    if b % 2 == 0:
                        st_map[p] = {"d": D_H,
                                     "sums": sp.tile([128, 2], F32,
                                                     tag="sums",
                                                     name="sums"),
                                     "h": [h_sb[:, 2 * p, :],
                                           h_sb[:, 2 * p + 1, :]]}
                    st = st_map[p]
                    t4 = wp.tile([128, D_H], F32, tag="t4")
                    nc.vector.scalar_tensor_tensor(
                        t4[:], ps[:], dv[:, b:b + 1], bias[:],
                        OP.mult, OP.add)
                    # h = relu(t4) + h_old, accumulating LN row sums
                    nc.vector.scalar_tensor_tensor(
                        h_sb[:, b, :], t4[:], 0.0, h_sb[:, b, :],
                        OP.max, OP.add, accum_out=st["sums"][:, b % 2:
                                                             b % 2 + 1])
                return passA

            def mk_mid_stage1(st_map, zs_map):
                def stage1(p):
                    st = st_map.pop(p)
                    pair_stats(p, st)
                    s, negmu_s = ln_scale_pair(p, st)
                    zs = []
                    for k in (0, 1):
                        z = wp.tile([128, D_H], BF, tag="z")
                        nc.scalar.activation(z[:], st["h"][k], AF.Identity,
                                             bias=negmu_s[:, k:k + 1],
                                             scale=s[:, k:k + 1])
                        zs.append(z)
                    zs_map[p] = zs
                return stage1

            def mk_mid_stage2(zs_map, w, brow):
                def stage2(p):
                    store_u_pair(p, w, brow, ul23_bf, zs_map.pop(p))
                return stage2

            l2_st, l2_zs = {}, {}
            _l2s1 = mk_mid_stage1(l2_st, l2_zs)
            _l2s2 = mk_mid_stage2(l2_zs, w2, brow2)
            layer_loop(uf2, D_H, F8, ul23_bf, mk_mid(b1, l2_st),
                       lambda p: (_l2s1(p), _l2s2(p)), None,
                       cc_map={
                           NBH // 2: lambda: allgather_half(ul23_f8, uf3, 0)})
            allgather_half(ul23_f8, uf3, 1)

            l3_st, l3_zs = {}, {}

            def l3_stage1(p):
                # u-compute for layer 4: (h * dinv) @ W3  (no LN)
                st = l3_st.pop(p)
                zs = []
                for k in (0, 1):
                    b = 2 * p + k
                    z = wp.tile([128, D_H], BF, tag="z")
                    nc.vector.tensor_scalar_mul(z[:], st["h"][k],
                                                dv[:, b:b + 1])
                    zs.append(z)
                l3_zs[p] = zs

            def l3_stage2(p):
                zs = l3_zs.pop(p)
                ubf = pp.tile([128, 2, D_OUT], BF, tag="ub4")
                for k in (0, 1):
                    mm = transpose_mm(zs[k], w3, D_OUT, 2, evac="S")
                    nc.scalar.activation(ubf[:, k, :], mm[:], AF.Copy)
                nc.sync.dma_start(r3(ul4, D_OUT)[:, 2 * p:2 * p + 2, :],
                                  ubf[:])

            layer_loop(uf3, D_H, F8, ul23_bf, mk_mid(b2, l3_st),
                       lambda p: (l3_stage1(p), l3_stage2(p)), None,
                       cc_map={
                           NBH // 2: lambda: allgather_half(ul4, uf4, 0)})
            allgather_half(ul4, uf4, 1)

            # ================= Layer 4 ====================================
            l4_st = {}

            def l4_passA(b, ps):
                p = b // 2
                if b % 2 == 0:
                    l4_st[p] = {"d": D_OUT,
                                "sums": sp.tile([128, 2], F32, tag="sums",
                                                name="sums"),
                                "y2": pp.tile([128, 2, D_OUT], F32,
                                              tag="y2", name="y2")}
                st = l4_st[p]
                nc.vector.scalar_tensor_tensor(
                    st["y2"][:, b % 2, :], ps[:], dv[:, b:b + 1], b3[:],
                    OP.mult, OP.add, accum_out=st["sums"][:, b % 2:b % 2 + 1])

            def l4_workP(p):
                st = l4_st.pop(p)
                st["h"] = [st["y2"][:, 0, :], st["y2"][:, 1, :]]
                pair_stats(p, st)
                zo = pp.tile([128, 2, D_OUT], F32, tag="zo")
                for k in (0, 1):
                    nc.vector.tensor_scalar(
                        zo[:, k, :], st["y2"][:, k, :],
                        st["negmu"][:, k:k + 1], st["rstd"][:, k:k + 1],
                        OP.add, OP.mult)
                    if fg is not None:
                        nc.vector.tensor_tensor(zo[:, k, :], zo[:, k, :],
                                                fg[:], OP.mult)
                    if fb is not None:
                        nc.vector.tensor_tensor(zo[:, k, :], zo[:, k, :],
                                                fb[:], OP.add)
                lo = 2 * p * 128
                if lo + 256 <= NLOC:
                    nc.sync.dma_start(
                        out_p[lo:lo + 256, :].rearrange(
                            "(n p) d -> p n d", p=128), zo[:])
                else:
                    for k in (0, 1):
                        l2_ = lo + k * 128
                        nrow = min(128, NLOC - l2_)
                        if nrow > 0:
                            nc.sync.dma_start(out_p[l2_:l2_ + nrow, :],
                                              zo[0:nrow, k, :])

            layer_loop(uf4, D_OUT, BF, ul4, l4_passA, l4_workP, None)

    nc.compile()
    return nc


_CACHE = {}


def kernel(x, edge_index, W0, b0, W1, b1, W2, b2, W3, b3,
           ln0_g, ln0_b, ln1_g, ln1_b, fln_g, fln_b):
    x = np.asarray(x, np.float32)
    edge_index = np.asarray(edge_index)
    (gidx, Ss, calls, block_groups, need_call, M, Gtot,
     dinv) = _prep_graph(edge_index)

    W1f = np.asarray(ln0_g, np.float32)[:, None] * np.asarray(W1, np.float32)
    W2f = np.asarray(ln1_g, np.float32)[:, None] * np.asarray(W2, np.float32)
    brow1 = np.asarray(ln0_b, np.float32) @ np.asarray(W1, np.float32)
    brow2 = np.asarray(ln1_b, np.float32) @ np.asarray(W2, np.float32)
    use_brow1 = bool(np.any(brow1 != 0))
    use_brow2 = bool(np.any(brow2 != 0))
    use_fg = bool(np.any(np.asarray(fln_g) != 1))
    use_fb = bool(np.any(np.asarray(fln_b) != 0))

    key = (M, Gtot, tuple(calls), tuple(need_call),
           tuple(tuple(g) for g in block_groups),
           use_brow1, use_brow2, use_fg, use_fb)
    if key not in _CACHE:
        _CACHE[key] = _build(M, Gtot, calls, block_groups, need_call,
                             use_brow1, use_brow2, use_fg, use_fb)
    nc = _CACHE[key]

    u0 = dinv[:, None].astype(np.float32) * x
    u0p = np.zeros((NCORES, NPAD, D_IN), BF16)
    for r in range(NCORES):
        u0p[r, :NLOC] = u0[r * NLOC:(r + 1) * NLOC]
    dinv_pad = np.zeros((NCORES, NPAD), np.float32)
    for r in range(NCORES):
        dinv_pad[r, :NLOC] = dinv[r * NLOC:(r + 1) * NLOC]

    def chunk2(Wf):
        return np.stack([Wf[0:128], Wf[128:256]]).astype(BF16)

    common = {
        "ident": np.eye(128, dtype=BF16),
        "w0": np.asarray(W0, np.float32).astype(BF16),
        "w1": chunk2(W1f), "w2": chunk2(W2f),
        "w3": chunk2(np.asarray(W3, np.float32)),
        "b0r": _rep(b0), "b1r": _rep(b1), "b2r": _rep(b2), "b3r": _rep(b3),
    }
    if use_brow1:
        common["brow1r"] = _rep(brow1)
    if use_brow2:
        common["brow2r"] = _rep(brow2)
    if use_fg:
        common["fgr"] = _rep(fln_g)
    if use_fb:
        common["fbr"] = _rep(fln_b)

    in_maps = []
    for r in range(NCORES):
        m = dict(common)
        m["u0_own"] = u0p[r]
        m["gidx"] = gidx[r]
        m["S"] = Ss[r]
        m["dinv"] = np.ascontiguousarray(dinv_pad[r].reshape(NT, 128).T)
        in_maps.append(m)

    res = run_bass_kernel_spmd(nc, in_maps, core_ids=list(range(NCORES)))
    out = np.concatenate([res.results[r]["out"] for r in range(NCORES)],
                         axis=0)
    return out.astype(np.float32)


# revision 19
# speedup vs baseline: 1.0337x; 1.0003x over previous
"""4-layer GCN (ArithmeticCircuitGNN) on 8 Trainium2 NeuronCores.

Node-parallel, aggregation on the TENSOR engine: 12544-padded shard/core,
LN affine folded into weights on host.  Per GCN layer:
  AllGather(u) -> dma_gather(u[src]) in (chunk, dst-block) cells ->
  one-hot matmul S^T @ msg accumulated in PSUM per dst block, seeded with
  the self-loop term via an identity matmul -> evac (*dinv_dst, +bias,
  relu, +residual).  No DRAM scatter-add, no V planes.
Each AllGather is split in two shard-row halves: the first half fires
mid-way through the previous layer's block loop and hides under compute.
The one-hot S tiles (fp8, 1.0 entries) and the gather index table are
shared by all 4 layers (same graph).

v2: the two d_h=256 layers (uf2/uf3) carry fp8 messages on the wire and
through the gather + one-hot matmuls (self-loop seed stays bf16 via a
separate bf16 copy of u); relu+residual+mean-accum fused into one
scalar_tensor_tensor.

v3: software-pipelined block loop.  Each block's work is split into
pass A (aggregate + PSUM evac into h, accumulating LN row sums) and
pass B (LN stats -> z -> W matmul -> u stores), with pass B emitted
DELTA blocks behind pass A so every engine queue always holds ready
work (the in-order sequencers otherwise serialize the ~14-op
cross-engine chain per block).  LN stats are batched over block pairs
and u/vt/out DMAs cover two blocks each (dma_start issue on the sync
sequencer costs ~0.8us).

kernel(**inputs) takes FULL numpy inputs, returns FULL [100000,128] out.
"""

import os
import numpy as np
import ml_dtypes

import concourse.bass as bass
import concourse.bacc as bacc
import concourse.mybir as mybir
import concourse.tile as tile
from concourse.bass_utils import run_bass_kernel_spmd

BF16 = ml_dtypes.bfloat16
FP8 = ml_dtypes.float8_e4m3

N = 100000
E = 300000
NCORES = 8
NLOC = 12500
NPAD = 12544          # 98 * 128
NT = 98
HALF = NPAD // 2      # 6272 = 49 * 128 (collective split point)
NBH = NT // 2         # 49 blocks per half
NG = NPAD * NCORES    # 100352
NGH = HALF * NCORES   # 50176 rows per uf half-tensor
CHUNK = HALF * 4      # 25088 rows (4 ranks' halves) per gather chunk
NCHUNK = 4            # A0 A1 B0 B1
D_IN, D_H, D_OUT = 128, 256, 128
EPS = 1e-5
CALLG = 8             # groups per gather call (8*128 = 1024 idxs, ring cap)
NQ = 4                # SWDGE queues
VT_AHEAD = 4          # self-term DMA prefetch distance (in PAIRS)
DELTA = 5             # pass-B lag behind pass A, in blocks
GAH = 6               # gather emission lookahead, in blocks

F32 = mybir.dt.float32
BF = mybir.dt.bfloat16
I16 = mybir.dt.int16
F8 = mybir.dt.float8e4

SKIP_AGG = bool(int(os.environ.get("KERNEL_SKIP_AGG", "0")))
SKIP_CC = bool(int(os.environ.get("KERNEL_SKIP_CC", "0")))


# ---------------------------------------------------------------- host prep

def _wrap16(idx):
    """[M] -> [128, M//16]: position i -> (i%16, i//16), replicated x8."""
    M = len(idx)
    w = np.zeros((128, M // 16), dtype=np.int16)
    t = idx.reshape(M // 16, 16).T
    for g in range(8):
        w[g * 16:(g + 1) * 16, :] = t
    return w


def _prep_graph(edge_index):
    """Cell (chunk, dst-block) schedule for one-hot-matmul aggregation.

    Chunks: A0/A1 = first shard-halves of ranks 0-3 / 4-7, B0/B1 = second
    halves, matching the split AllGather output tensors ufA/ufB.
    Returns (gidx[8], S[8], calls, block_groups, need_call, M, Gtot, dinv);
    calls = [(chunk, row_off, n_rows), ...] in emission order;
    block_groups[b] = [(call_idx, slot, G), ...];  schedule uniform across
    cores (group counts maxed over cores)."""
    src = np.asarray(edge_index[0], dtype=np.int64)
    dst = np.asarray(edge_index[1], dtype=np.int64)
    deg = np.bincount(dst, minlength=N).astype(np.float64) + 1.0
    dinv = (1.0 / np.sqrt(deg)).astype(np.float32)

    r_arr = dst // NLOC
    dloc = dst - r_arr * NLOC
    b_arr = dloc // 128
    drow = dloc - b_arr * 128
    srank = src // NLOC
    spad = src % NLOC                      # row within shard (pad ignored)
    half = (spad >= HALF).astype(np.int64)
    quad = srank // 4
    c_arr = half * 2 + quad
    crow = (srank % 4) * HALF + spad - half * HALF

    counts = np.bincount(
        (r_arr * NCHUNK + c_arr) * NT + b_arr,
        minlength=NCORES * NCHUNK * NT).reshape(NCORES, NCHUNK, NT)
    k = -(-counts.max(axis=0) // 128)        # [NCHUNK, NT] groups per cell
    ngroups = k.sum(axis=1)                  # per chunk
    base = np.zeros((NCHUNK, NT), np.int64)  # group base within chunk
    base[:, 1:] = np.cumsum(k, axis=1)[:, :-1]

    ncalls = [-(-int(g) // CALLG) for g in ngroups]
    merged = []                              # (chunk, local_call)
    for i in range(max(ncalls)):
        for c in range(NCHUNK):
            if i < ncalls[c]:
                merged.append((c, i))
    calls = []
    cidx = {}
    goff = {}                                # (c, local_call) -> global G base
    off = 0
    for ci, (c, i) in enumerate(merged):
        g0 = i * CALLG
        ng = min(CALLG, int(ngroups[c]) - g0)
        calls.append((c, off, ng * 128))
        cidx[(c, i)] = ci
        goff[(c, i)] = off // 128
        off += ng * 128
    M = off
    Gtot = M // 128

    # gmap[c, g_loc] -> global group id
    gmap = np.zeros((NCHUNK, max(1, int(ngroups.max()))), np.int64)
    callof = np.zeros_like(gmap)
    for c in range(NCHUNK):
        for g in range(int(ngroups[c])):
            i = g // CALLG
            gmap[c, g] = goff[(c, i)] + g % CALLG
            callof[c, g] = cidx[(c, i)]

    block_groups = [[] for _ in range(NT)]
    need_call = np.full(NT, -1, np.int64)
    for b in range(NT):
        for c in range(NCHUNK):
            for j in range(int(k[c, b])):
                g = int(base[c, b]) + j
                ci = int(callof[c, g])
                block_groups[b].append((ci, g % CALLG, int(gmap[c, g])))
                need_call[b] = max(need_call[b], ci)

    gidx, Ss = [], []
    for r in range(NCORES):
        m = r_arr == r
        ec, eb = c_arr[m], b_arr[m]
        ecrow, edrow = crow[m], drow[m]
        cell = ec * NT + eb
        order = np.argsort(cell, kind="stable")
        cell_s = cell[order]
        starts = np.searchsorted(cell_s, np.arange(NCHUNK * NT))
        tpos = np.arange(len(cell_s)) - starts[cell_s]
        g_loc = base.reshape(-1)[cell_s] + tpos // 128
        G = gmap[cell_s // NT, g_loc]
        row = G * 128 + tpos % 128
        g_rows = np.zeros(M, np.int16)
        g_rows[row] = ecrow[order]
        S3 = np.zeros((128, Gtot, 128), FP8)
        S3[tpos % 128, G, edrow[order]] = 1.0
        gidx.append(_wrap16(g_rows))
        Ss.append(np.ascontiguousarray(S3.reshape(128, Gtot * 128)))
    return gidx, Ss, calls, block_groups, need_call, M, Gtot, dinv


def _rep(v, p=128):
    return np.ascontiguousarray(
        np.broadcast_to(np.asarray(v, np.float32), (p, len(v))))


MP_BUFS = 6


def _check_liveness(calls, block_groups, need_call):
    """Every matmul must read a msg tile within the last MP_BUFS of its
    chunk's pool tag, else pool rotation clobbers it.  Emission follows
    the GAH-block lookahead used by layer_loop."""
    emitted = {c: [] for c in range(NCHUNK)}
    ci = 0
    for b in range(NT):
        while ci <= need_call[min(b + GAH, NT - 1)]:
            emitted[calls[ci][0]].append(ci)
            ci += 1
        for (cj, slot, G) in block_groups[b]:
            c = calls[cj][0]
            assert cj in emitted[c][-MP_BUFS:], (
                f"block {b} reads call {cj} beyond pool depth "
                f"{emitted[c][-MP_BUFS - 2:]}")


# ---------------------------------------------------------------- builder

def _build(M, Gtot, calls, block_groups, need_call,
           use_brow1, use_brow2, use_fg, use_fb):
    _check_liveness(calls, block_groups, need_call)
    nc = bacc.Bacc(None, target_bir_lowering=False, num_swdge_queues=NQ)

    def param(name, shape, dt, out=False):
        return nc.declare_dram_parameter(name, shape, dt, isOutput=out)

    u0_own = param("u0_own", [NPAD, D_IN], BF)
    gidx = param("gidx", [128, M // 16], I16)
    S_p = param("S", [128, Gtot * 128], F8)
    dinv_p = param("dinv", [128, NT], F32)
    ident_p = param("ident", [128, 128], BF)
    w0_p = param("w0", [128, D_H], BF)
    w1_p = param("w1", [2, 128, D_H], BF)
    w2_p = param("w2", [2, 128, D_H], BF)
    w3_p = param("w3", [2, 128, D_OUT], BF)
    b0_p = param("b0r", [128, D_H], F32)
    b1_p = param("b1r", [128, D_H], F32)
    b2_p = param("b2r", [128, D_H], F32)
    b3_p = param("b3r", [128, D_OUT], F32)
    brow1_p = param("brow1r", [128, D_H], F32) if use_brow1 else None
    brow2_p = param("brow2r", [128, D_H], F32) if use_brow2 else None
    fg_p = param("fgr", [128, D_OUT], F32) if use_fg else None
    fb_p = param("fbr", [128, D_OUT], F32) if use_fb else None
    out_p = param("out", [NLOC, D_OUT], F32, out=True)

    ul0 = nc.dram_tensor("ul0", [NPAD, D_IN], BF)
    ul23_bf = nc.dram_tensor("ul23_bf", [NPAD, D_H], BF)   # self-term source
    ul23_f8 = nc.dram_tensor("ul23_f8", [NPAD, D_H], F8)   # wire source
    ul4 = nc.dram_tensor("ul4", [NPAD, D_OUT], BF)
    uf0 = [nc.dram_tensor(f"uf0{h}", [NGH, D_IN], BF, addr_space="Shared")
           for h in "AB"]
    uf2 = [nc.dram_tensor(f"uf2{h}", [NGH, D_H], F8, addr_space="Shared")
           for h in "AB"]
    uf3 = [nc.dram_tensor(f"uf3{h}", [NGH, D_H], F8, addr_space="Shared")
           for h in "AB"]
    uf4 = [nc.dram_tensor(f"uf4{h}", [NGH, D_OUT], BF, addr_space="Shared")
           for h in "AB"]

    AX = mybir.AxisListType.X
    AF = mybir.ActivationFunctionType
    OP = mybir.AluOpType

    with tile.TileContext(nc) as tc:
        with (
            tc.tile_pool(name="const", bufs=1) as cp,
            tc.tile_pool(name="hbuf", bufs=1) as hp,
            tc.tile_pool(name="work", bufs=4) as wp,
            tc.tile_pool(name="sqp", bufs=2) as qp,
            tc.tile_pool(name="pair", bufs=4) as pp,
            tc.tile_pool(name="vtp", bufs=VT_AHEAD + 2) as vp,
            tc.tile_pool(name="small", bufs=6) as sp,
            tc.tile_pool(name="msg", bufs=MP_BUFS) as mp,
            tc.tile_pool(name="psT", bufs=2, space="PSUM") as pT,
            tc.tile_pool(name="psM", bufs=2, space="PSUM") as pM,
            tc.tile_pool(name="psA", bufs=4, space="PSUM") as pA,
        ):
            def cload(par, shape, dt):
                t = cp.tile(shape, dt, tag=par.name)
                nc.sync.dma_start(t[:], par[:])
                return t

            def allgather_half(ul, uf, h):
                if SKIP_CC:
                    return
                nc.gpsimd.collective_compute(
                    "AllGather", OP.bypass,
                    ins=[ul[h * HALF:(h + 1) * HALF, :].opt()],
                    outs=[uf[h][:].opt()],
                    replica_groups=[list(range(NCORES))],
                )

            QTR = HALF // 2

            def allgather_b_quarter(ul, uf, q):
                """Quarter q of the second shard-half; output rows are
                rank-strided slices of uf[1] (rank r rows at r*HALF+q*QTR)."""
                if SKIP_CC:
                    return
                d = ul.shape[1]
                ufv = uf[1].rearrange("(r q x) d -> r q (x d)", r=NCORES, q=2)
                nc.gpsimd.collective_compute(
                    "AllGather", OP.bypass,
                    ins=[ul[HALF + q * QTR:HALF + (q + 1) * QTR, :].opt()],
                    outs=[ufv[:, q, :].opt()],
                    replica_groups=[list(range(NCORES))],
                )

            # Layer-1 collectives first (input staged to ul0: collectives
            # can't read IO tensors); they overlap the constant loads below.
            nc.sync.dma_start(ul0[0:HALF, :], u0_own[0:HALF, :])
            allgather_half(ul0, uf0, 0)
            nc.sync.dma_start(ul0[HALF:NPAD, :], u0_own[HALF:NPAD, :])
            allgather_half(ul0, uf0, 1)

            gi = cload(gidx, [128, M // 16], I16)
            S_sb = cload(S_p, [128, Gtot * 128], F8)
            S_v = S_sb.rearrange("p (g d) -> p g d", d=128)
            dv = cload(dinv_p, [128, NT], F32)
            idn = cload(ident_p, [128, 128], BF)
            w0 = cload(w0_p, [128, D_H], BF)

            def wload(par, d):
                t = cp.tile([128, 2, d], BF, tag=par.name)
                nc.sync.dma_start(t[:], par.rearrange("k p d -> p k d"))
                return t

            w1 = wload(w1_p, D_H)
            w2 = wload(w2_p, D_H)
            w3 = wload(w3_p, D_OUT)
            b0 = cload(b0_p, [128, D_H], F32)
            b1 = cload(b1_p, [128, D_H], F32)
            b2 = cload(b2_p, [128, D_H], F32)
            b3 = cload(b3_p, [128, D_OUT], F32)
            brow1 = cload(brow1_p, [128, D_H], F32) if use_brow1 else None
            brow2 = cload(brow2_p, [128, D_H], F32) if use_brow2 else None
            fg = cload(fg_p, [128, D_OUT], F32) if use_fg else None
            fb = cload(fb_p, [128, D_OUT], F32) if use_fb else None

            h_sb = hp.tile([128, NT, D_H], BF)

            def r3(t, d):
                return t.rearrange("(n p) d -> p n d", p=128)

            def emit_gather(ci, uf, d, dt):
                """One gather call -> flat msg tile; returns [128,G,d] view."""
                (c, off, n) = calls[ci]
                t = mp.tile([128, CALLG * d], dt, tag=f"m{c}")
                tv = t.rearrange("p (g d) -> p g d", d=d)
                src = uf[c // 2][(c % 2) * CHUNK:(c % 2 + 1) * CHUNK, :]
                nc.gpsimd.dma_gather(
                    tv[:, : n // 128, :], src,
                    gi[:, off // 16:(off + n) // 16], n, n, d,
                    queue_num=ci % NQ,
                )
                return tv

            def agg_block(b, tiles, vt, d):
                """Self-term seed + one-hot matmuls for block b -> psum."""
                ps = pA.tile([128, d], F32, tag="agg")
                groups = [] if SKIP_AGG else block_groups[b]
                nc.tensor.matmul(ps[:], idn[:], vt[:],
                                 start=True, stop=not groups)
                for j, (ci, slot, G) in enumerate(groups):
                    nc.tensor.matmul(ps[:], S_v[:, G, :], tiles[ci][:, slot, :],
                                     start=False, stop=(j == len(groups) - 1))
                return ps

            def transpose_mm(z_bf, w, d_out, kchunks, evac):
                """z_bf [128, kchunks*128] -> mm = z^T @ w in PSUM.
                evac: 'S' or 'V' engine for the transpose evacuation."""
                mm = pM.tile([128, d_out], F32, tag="mm")
                zt_ps = pT.tile([128, kchunks, 128], BF, tag="zt_ps")
                for kk in range(kchunks):
                    nc.tensor.transpose(
                        zt_ps[:, kk, :], z_bf[:, kk * 128:(kk + 1) * 128],
                        idn[:])
                zt = wp.tile([128, kchunks, 128], BF, tag="zt")
                if evac == "S":
                    nc.scalar.activation(zt[:], zt_ps[:], AF.Copy)
                else:
                    nc.vector.tensor_scalar_mul(zt[:], zt_ps[:], 1.0)
                for kk in range(kchunks):
                    nc.tensor.matmul(mm[:], zt[:, kk, :],
                                     w[:, kk, :] if kchunks > 1 else w[:],
                                     start=(kk == 0), stop=(kk == kchunks - 1))
                return mm

            # ---------------- LN stats over a block pair ------------------
            def pair_stats(p, st):
                """Consumes st['sums'] [128,2]; fills negmu/s/negmu_s [128,2].
                Variance via Square+accum per block (bias = per-block
                negmu), then batched sqrt/recip/scale over the pair."""
                d = st["d"]
                negmu = sp.tile([128, 2], F32, tag="negmu")
                nc.vector.tensor_scalar_mul(negmu[:], st["sums"][:], -1.0 / d)
                ssq = sp.tile([128, 2], F32, tag="ssq")
                for k in (0, 1):
                    sq = qp.tile([128, d], BF, tag="sq")
                    nc.scalar.activation(sq[:], st["h"][k], AF.Square,
                                         bias=negmu[:, k:k + 1],
                                         accum_out=ssq[:, k:k + 1])
                varp = sp.tile([128, 2], F32, tag="varp")
                nc.vector.tensor_scalar(varp[:], ssq[:], 1.0 / d, EPS,
                                        OP.mult, OP.add)
                sd = sp.tile([128, 2], F32, tag="sd")
                nc.scalar.sqrt(sd[:], varp[:])
                rstd = sp.tile([128, 2], F32, tag="rstd")
                nc.vector.reciprocal(rstd[:], sd[:])
                st["negmu"] = negmu
                st["rstd"] = rstd
                return negmu, rstd

            def ln_scale_pair(p, st):
                """s = rstd*dinv, negmu_s = negmu*s for the pair."""
                s = sp.tile([128, 2], F32, tag="s")
                nc.vector.tensor_tensor(s[:], st["rstd"][:],
                                        dv[:, 2 * p:2 * p + 2], OP.mult)
                negmu_s = sp.tile([128, 2], F32, tag="negmu_s")
                nc.vector.tensor_tensor(negmu_s[:], st["negmu"][:], s[:],
                                        OP.mult)
                return s, negmu_s

            def store_u_pair(p, w, brow, ul_bf, zs):
                """z (bf16) per block -> mm -> u bf16+fp8, pair-batched DMA."""
                ubf = pp.tile([128, 2, D_H], BF, tag="ubf")
                u8 = pp.tile([128, 2, D_H], F8, tag="u8")
                for k in (0, 1):
                    mm = transpose_mm(zs[k], w, D_H, 2, evac="S")
                    if brow is not None:
                        b = 2 * p + k
                        nc.vector.scalar_tensor_tensor(
                            ubf[:, k, :], brow[:], dv[:, b:b + 1], mm[:],
                            OP.mult, OP.add)
                    else:
                        nc.vector.tensor_scalar_mul(ubf[:, k, :], mm[:], 1.0)
                    nc.vector.tensor_scalar_mul(u8[:, k, :], ubf[:, k, :],
                                                1.0)
                nc.sync.dma_start(r3(ul_bf, D_H)[:, 2 * p:2 * p + 2, :],
                                  ubf[:])
                nc.sync.dma_start(r3(ul23_f8, D_H)[:, 2 * p:2 * p + 2, :],
                                  u8[:])

            # ---------------- pipelined layer loop ------------------------
            def layer_loop(uf, d, dt, ul_self, passA, stage1, stage2,
                           cc_map=None):
                """passA(b, ps): immediate PSUM evac (writes h/sums).
                stage1(p)/stage2(p): pass-B pair stages, stage1 lagging
                DELTA blocks behind pass A and stage2 one pair-tick
                behind stage1 (so stage2's PE work never waits on fresh
                stage1 results).  Stages are emitted oldest-first within
                an iteration to keep each engine queue head ready.
                mid_cc fires right after stage2(mid_pair)."""
                tiles = {}
                vts = {}
                ci = 0
                lag2 = DELTA + 2 if stage2 is not None else None
                cc_map = cc_map or {}

                def vt_dma(pv):
                    vt = vp.tile([128, 2, d], BF, tag="vt")
                    nc.sync.dma_start(vt[:],
                                      r3(ul_self, d)[:, 2 * pv:2 * pv + 2, :])
                    vts[pv] = vt

                def stage(fn, b, is_last):
                    if fn is not None and 0 <= b < NT and b % 2 == 1:
                        pr = b // 2
                        fn(pr)
                        if is_last and pr in cc_map:
                            cc_map[pr]()

                for pv in range(min(VT_AHEAD, NBH)):
                    vt_dma(pv)
                for b in range(NT):
                    while ci <= need_call[min(b + GAH, NT - 1)]:
                        tiles[ci] = emit_gather(ci, uf, d, dt)
                        ci += 1
                    pv = b // 2 + VT_AHEAD
                    if b % 2 == 0 and pv < NBH:
                        vt_dma(pv)
                    if stage2 is not None:
                        stage(stage2, b - lag2, True)
                        stage(stage1, b - DELTA, False)
                    else:
                        stage(stage1, b - DELTA, True)
                    ps = agg_block(b, tiles, vts[b // 2][:, b % 2, :], d)
                    passA(b, ps)
                end = NT + (lag2 if stage2 is not None else DELTA)
                for b in range(NT, end):
                    if stage2 is not None:
                        stage(stage2, b - lag2, True)
                        stage(stage1, b - DELTA, False)
                    else:
                        stage(stage1, b - DELTA, True)

            # ================= Layer 1 ====================================
            l1_st = {}

            def l1_passA(b, ps):
                p = b // 2
                if b % 2 == 0:
                    l1_st[p] = {"d": D_H,
                                "tbf": pp.tile([128, 2, D_IN], BF,
                                               tag="tbf", name="tbf")}
                nc.scalar.activation(l1_st[p]["tbf"][:, b % 2, :], ps[:],
                                     AF.Copy)

            l1_zs = {}

            def l1_stage1(p):
                st = l1_st.pop(p)
                st["sums"] = sp.tile([128, 2], F32, tag="sums",
                                     name="sums")
                st["h"] = [h_sb[:, 2 * p, :], h_sb[:, 2 * p + 1, :]]
                for k in (0, 1):
                    b = 2 * p + k
                    mm = transpose_mm(st["tbf"][:, k, :], w0, D_H, 1,
                                      evac="V")
                    t2 = wp.tile([128, D_H], F32, tag="t2")
                    nc.vector.scalar_tensor_tensor(
                        t2[:], mm[:], dv[:, b:b + 1], b0[:], OP.mult, OP.add)
                    # relu with row-sum accum, on vector (op1 bypass)
                    nc.vector.scalar_tensor_tensor(
                        h_sb[:, b, :], t2[:], 0.0, t2[:], OP.max, OP.bypass,
                        accum_out=st["sums"][:, k:k + 1])
                pair_stats(p, st)
                s, negmu_s = ln_scale_pair(p, st)
                zs = []
                for k in (0, 1):
                    z = wp.tile([128, D_H], BF, tag="z")
                    nc.scalar.activation(z[:], st["h"][k], AF.Identity,
                                         bias=negmu_s[:, k:k + 1],
                                         scale=s[:, k:k + 1])
                    zs.append(z)
                l1_zs[p] = zs

            def l1_stage2(p):
                store_u_pair(p, w1, brow1, ul23_bf, l1_zs.pop(p))

            layer_loop(uf0, D_IN, BF, u0_own, l1_passA,
                       lambda p: (l1_stage1(p), l1_stage2(p)), None,
                       cc_map={
                           NBH // 2: lambda: allgather_half(ul23_f8, uf2, 0)})
            allgather_half(ul23_f8, uf2, 1)

            # ================= Layers 2, 3 ================================
            def mk_mid(bias, st_map):
                def passA(b, ps):
                    p = b // 2
                    if b % 2 == 0:
                        st_map[p] = {"d": D_H,
                                     "sums": sp.tile([128, 2], F32,
                                                     tag="sums",
                                                     name="sums"),
                                     "h": [h_sb[:, 2 * p, :],
                                           h_sb[:, 2 * p + 1, :]]}
                    st = st_map[p]
                    t4 = wp.tile([128, D_H], F32, tag="t4")
                    nc.vector.scalar_tensor_tensor(
                        t4[:], ps[:], dv[:, b:b + 1], bias[:],
                        OP.mult, OP.add)
                    # h = relu(t4) + h_old, accumulating LN row sums
                    nc.vector.scalar_tensor_tensor(
                        h_sb[:, b, :], t4[:], 0.0, h_sb[:, b, :],
                        OP.max, OP.add, accum_out=st["sums"][:, b % 2:
                                                             b % 2 + 1])
                return passA

            def mk_mid_stage1(st_map, zs_map):
                def stage1(p):
                    st = st_map.pop(p)
                    pair_stats(p, st)
                    s, negmu_s = ln_scale_pair(p, st)
                    zs = []
                    for k in (0, 1):
                        z = wp.tile([128, D_H], BF, tag="z")
                        nc.scalar.activation(z[:], st["h"][k], AF.Identity,
                                             bias=negmu_s[:, k:k + 1],
                                             scale=s[:, k:k + 1])
                        zs.append(z)
                    zs_map[p] = zs
                return stage1

            def mk_mid_stage2(zs_map, w, brow):
                def stage2(p):
                    store_u_pair(p, w, brow, ul23_bf, zs_map.pop(p))
                return stage2

            l2_st, l2_zs = {}, {}
            _l2s1 = mk_mid_stage1(l2_st, l2_zs)
            _l2s2 = mk_mid_stage2(l2_zs, w2, brow2)
            layer_loop(uf2, D_H, F8, ul23_bf, mk_mid(b1, l2_st),
                       lambda p: (_l2s1(p), _l2s2(p)), None,
                       cc_map={
                           NBH // 2: lambda: allgather_half(ul23_f8, uf3, 0)})
            allgather_half(ul23_f8, uf3, 1)

            l3_st, l3_zs = {}, {}

            def l3_stage1(p):
                # u-compute for layer 4: (h * dinv) @ W3  (no LN)
                st = l3_st.pop(p)
                zs = []
                for k in (0, 1):
                    b = 2 * p + k
                    z = wp.tile([128, D_H], BF, tag="z")
                    nc.vector.tensor_scalar_mul(z[:], st["h"][k],
                                                dv[:, b:b + 1])
                    zs.append(z)
                l3_zs[p] = zs

            def l3_stage2(p):
                zs = l3_zs.pop(p)
                ubf = pp.tile([128, 2, D_OUT], BF, tag="ub4")
                for k in (0, 1):
                    mm = transpose_mm(zs[k], w3, D_OUT, 2, evac="S")
                    nc.scalar.activation(ubf[:, k, :], mm[:], AF.Copy)
                nc.sync.dma_start(r3(ul4, D_OUT)[:, 2 * p:2 * p + 2, :],
                                  ubf[:])

            layer_loop(uf3, D_H, F8, ul23_bf, mk_mid(b2, l3_st),
                       lambda p: (l3_stage1(p), l3_stage2(p)), None,
                       cc_map={
                           NBH // 2: lambda: allgather_half(ul4, uf4, 0)})
            allgather_half(ul4, uf4, 1)

            # ================= Layer 4 ====================================
            l4_st = {}

            def l4_passA(b, ps):
                p = b // 2
                if b % 2 == 0:
                    l4_st[p] = {"d": D_OUT,
                                "sums": sp.tile([128, 2], F32, tag="sums",
                                                name="sums"),
                                "y2": pp.tile([128, 2, D_OUT], F32,
                                              tag="y2", name="y2")}
                st = l4_st[p]
                nc.vector.scalar_tensor_tensor(
                    st["y2"][:, b % 2, :], ps[:], dv[:, b:b + 1], b3[:],
                    OP.mult, OP.add, accum_out=st["sums"][:, b % 2:b % 2 + 1])

            def l4_workP(p):
                st = l4_st.pop(p)
                st["h"] = [st["y2"][:, 0, :], st["y2"][:, 1, :]]
                pair_stats(p, st)
                zo = pp.tile([128, 2, D_OUT], F32, tag="zo")
                for k in (0, 1):
                    nc.vector.tensor_scalar(
                        zo[:, k, :], st["y2"][:, k, :],
                        st["negmu"][:, k:k + 1], st["rstd"][:, k:k + 1],
                        OP.add, OP.mult)
                    if fg is not None:
                        nc.vector.tensor_tensor(zo[:, k, :], zo[:, k, :],
                                                fg[:], OP.mult)
                    if fb is not None:
                        nc.vector.tensor_tensor(zo[:, k, :], zo[:, k, :],
                                                fb[:], OP.add)
                lo = 2 * p * 128
                if lo + 256 <= NLOC:
                    nc.sync.dma_start(
                        out_p[lo:lo + 256, :].rearrange(
                            "(n p) d -> p n d", p=128), zo[:])
                else:
                    for k in (0, 1):
                        l2_ = lo + k * 128
                        nrow = min(128, NLOC - l2_)
                        if nrow > 0:
                            nc.sync.dma_start(out_p[l2_:l2_ + nrow, :],
                                              zo[0:nrow, k, :])

            layer_loop(uf4, D_OUT, BF, ul4, l4_passA, l4_workP, None)

    nc.compile()
    return nc


_CACHE = {}


def kernel(x, edge_index, W0, b0, W1, b1, W2, b2, W3, b3,
           ln0_g, ln0_b, ln1_g, ln1_b, fln_g, fln_b):
    x = np.asarray(x, np.float32)
    edge_index = np.asarray(edge_index)
    (gidx, Ss, calls, block_groups, need_call, M, Gtot,
     dinv) = _prep_graph(edge_index)

    W1f = np.asarray(ln0_g, np.float32)[:, None] * np.asarray(W1, np.float32)
    W2f = np.asarray(ln1_g, np.float32)[:, None] * np.asarray(W2, np.float32)
    brow1 = np.asarray(ln0_b, np.float32) @ np.asarray(W1, np.float32)
    brow2 = np.asarray(ln1_b, np.float32) @ np.asarray(W2, np.float32)
    use_brow1 = bool(np.any(brow1 != 0))
    use_brow2 = bool(np.any(brow2 != 0))
    use_fg = bool(np.any(np.asarray(fln_g) != 1))
    use_fb = bool(np.any(np.asarray(fln_b) != 0))

    key = (M, Gtot, tuple(calls), tuple(need_call),
           tuple(tuple(g) for g in block_groups),
           use_brow1, use_brow2, use_fg, use_fb)
    if key not in _CACHE:
        _CACHE[key] = _build(M, Gtot, calls, block_groups, need_call,
                             use_brow1, use_brow2, use_fg, use_fb)
    nc = _CACHE[key]

    u0 = dinv[:, None].astype(np.float32) * x
    u0p = np.zeros((NCORES, NPAD, D_IN), BF16)
    for r in range(NCORES):
        u0p[r, :NLOC] = u0[r * NLOC:(r + 1) * NLOC]
    dinv_pad = np.zeros((NCORES, NPAD), np.float32)
    for r in range(NCORES):
        dinv_pad[r, :NLOC] = dinv[r * NLOC:(r + 1) * NLOC]

    def chunk2(Wf):
        return np.stack([Wf[0:128], Wf[128:256]]).astype(BF16)

    common = {
        "ident": np.eye(128, dtype=BF16),
        "w0": np.asarray(W0, np.float32).astype(BF16),
        "w1": chunk2(W1f), "w2": chunk2(W2f),
        "w3": chunk2(np.asarray(W3, np.float32)),
        "b0r": _rep(b0), "b1r": _rep(b1), "b2r": _rep(b2), "b3r": _rep(b3),
    }
    if use_brow1:
        common["brow1r"] = _rep(brow1)
    if use_brow2:
        common["brow2r"] = _rep(brow2)
    if use_fg:
        common["fgr"] = _rep(fln_g)
    if use_fb:
        common["fbr"] = _rep(fln_b)

    in_maps = []
    for r in range(NCORES):
        m = dict(common)
        m["u0_own"] = u0p[r]
        m["gidx"] = gidx[r]
        m["S"] = Ss[r]
        m["dinv"] = np.ascontiguousarray(dinv_pad[r].reshape(NT, 128).T)
        in_maps.append(m)

    res = run_bass_kernel_spmd(nc, in_maps, core_ids=list(range(NCORES)))
    out = np.concatenate([res.results[r]["out"] for r in range(NCORES)],
                         axis=0)
    return out.astype(np.float32)


# revision 20
# speedup vs baseline: 1.0668x; 1.0320x over previous
"""4-layer GCN (ArithmeticCircuitGNN) on 8 Trainium2 NeuronCores.

Node-parallel, aggregation on the TENSOR engine: 12544-padded shard/core,
LN affine folded into weights on host.  Per GCN layer:
  AllGather(u) -> dma_gather(u[src]) in (chunk, dst-block) cells ->
  one-hot matmul S^T @ msg accumulated in PSUM per dst block, seeded with
  the self-loop term via an identity matmul -> evac (*dinv_dst, +bias,
  relu, +residual).  No DRAM scatter-add, no V planes.
Each AllGather is split in two shard-row halves: the first half fires
mid-way through the previous layer's block loop and hides under compute.
The one-hot S tiles (fp8, 1.0 entries) and the gather index table are
shared by all 4 layers (same graph).

v2: the two d_h=256 layers (uf2/uf3) carry fp8 messages on the wire and
through the gather + one-hot matmuls (self-loop seed stays bf16 via a
separate bf16 copy of u); relu+residual+mean-accum fused into one
scalar_tensor_tensor.

v3: software-pipelined block loop.  Each block's work is split into
pass A (aggregate + PSUM evac into h, accumulating LN row sums) and
pass B (LN stats -> z -> W matmul -> u stores), with pass B emitted
DELTA blocks behind pass A so every engine queue always holds ready
work (the in-order sequencers otherwise serialize the ~14-op
cross-engine chain per block).  LN stats are batched over block pairs
and u/vt/out DMAs cover two blocks each (dma_start issue on the sync
sequencer costs ~0.8us).

kernel(**inputs) takes FULL numpy inputs, returns FULL [100000,128] out.
"""

import os
import numpy as np
import ml_dtypes

import concourse.bass as bass
import concourse.bacc as bacc
import concourse.mybir as mybir
import concourse.tile as tile
from concourse.bass_utils import run_bass_kernel_spmd

BF16 = ml_dtypes.bfloat16
FP8 = ml_dtypes.float8_e4m3

N = 100000
E = 300000
NCORES = 8
NLOC = 12500
NPAD = 12544          # 98 * 128
NT = 98
HALF = NPAD // 2      # 6272 = 49 * 128 (collective split point)
NBH = NT // 2         # 49 blocks per half
NG = NPAD * NCORES    # 100352
NGH = HALF * NCORES   # 50176 rows per uf half-tensor
CHUNK = HALF * 4      # 25088 rows (4 ranks' halves) per gather chunk
NCHUNK = 4            # A0 A1 B0 B1
D_IN, D_H, D_OUT = 128, 256, 128
EPS = 1e-5
CALLG = 4             # groups per gather call (4*128 = 512 idxs)
NQ = 4                # SWDGE queues
VT_AHEAD = 4          # self-term DMA prefetch distance (in PAIRS)
DELTA = 5             # pass-B lag behind pass A, in blocks
GAH = 6               # gather emission lookahead, in blocks

F32 = mybir.dt.float32
BF = mybir.dt.bfloat16
I16 = mybir.dt.int16
F8 = mybir.dt.float8e4

SKIP_AGG = bool(int(os.environ.get("KERNEL_SKIP_AGG", "0")))
SKIP_CC = bool(int(os.environ.get("KERNEL_SKIP_CC", "0")))


# ---------------------------------------------------------------- host prep

def _wrap16(idx):
    """[M] -> [128, M//16]: position i -> (i%16, i//16), replicated x8."""
    M = len(idx)
    w = np.zeros((128, M // 16), dtype=np.int16)
    t = idx.reshape(M // 16, 16).T
    for g in range(8):
        w[g * 16:(g + 1) * 16, :] = t
    return w


def _prep_graph(edge_index):
    """Cell (chunk, dst-block) schedule for one-hot-matmul aggregation.

    Chunks: A0/A1 = first shard-halves of ranks 0-3 / 4-7, B0/B1 = second
    halves, matching the split AllGather output tensors ufA/ufB.
    Returns (gidx[8], S[8], calls, block_groups, need_call, M, Gtot, dinv);
    calls = [(chunk, row_off, n_rows), ...] in emission order;
    block_groups[b] = [(call_idx, slot, G), ...];  schedule uniform across
    cores (group counts maxed over cores)."""
    src = np.asarray(edge_index[0], dtype=np.int64)
    dst = np.asarray(edge_index[1], dtype=np.int64)
    deg = np.bincount(dst, minlength=N).astype(np.float64) + 1.0
    dinv = (1.0 / np.sqrt(deg)).astype(np.float32)

    r_arr = dst // NLOC
    dloc = dst - r_arr * NLOC
    b_arr = dloc // 128
    drow = dloc - b_arr * 128
    srank = src // NLOC
    spad = src % NLOC                      # row within shard (pad ignored)
    half = (spad >= HALF).astype(np.int64)
    quad = srank // 4
    c_arr = half * 2 + quad
    crow = (srank % 4) * HALF + spad - half * HALF

    counts = np.bincount(
        (r_arr * NCHUNK + c_arr) * NT + b_arr,
        minlength=NCORES * NCHUNK * NT).reshape(NCORES, NCHUNK, NT)
    k = -(-counts.max(axis=0) // 128)        # [NCHUNK, NT] groups per cell
    ngroups = k.sum(axis=1)                  # per chunk
    base = np.zeros((NCHUNK, NT), np.int64)  # group base within chunk
    base[:, 1:] = np.cumsum(k, axis=1)[:, :-1]

    ncalls = [-(-int(g) // CALLG) for g in ngroups]
    merged = []                              # (chunk, local_call)
    for i in range(max(ncalls)):
        for c in range(NCHUNK):
            if i < ncalls[c]:
                merged.append((c, i))
    calls = []
    cidx = {}
    goff = {}                                # (c, local_call) -> global G base
    off = 0
    for ci, (c, i) in enumerate(merged):
        g0 = i * CALLG
        ng = min(CALLG, int(ngroups[c]) - g0)
        calls.append((c, off, ng * 128))
        cidx[(c, i)] = ci
        goff[(c, i)] = off // 128
        off += ng * 128
    M = off
    Gtot = M // 128

    # gmap[c, g_loc] -> global group id
    gmap = np.zeros((NCHUNK, max(1, int(ngroups.max()))), np.int64)
    callof = np.zeros_like(gmap)
    for c in range(NCHUNK):
        for g in range(int(ngroups[c])):
            i = g // CALLG
            gmap[c, g] = goff[(c, i)] + g % CALLG
            callof[c, g] = cidx[(c, i)]

    block_groups = [[] for _ in range(NT)]
    need_call = np.full(NT, -1, np.int64)
    for b in range(NT):
        for c in range(NCHUNK):
            for j in range(int(k[c, b])):
                g = int(base[c, b]) + j
                ci = int(callof[c, g])
                block_groups[b].append((ci, g % CALLG, int(gmap[c, g])))
                need_call[b] = max(need_call[b], ci)

    gidx, Ss = [], []
    for r in range(NCORES):
        m = r_arr == r
        ec, eb = c_arr[m], b_arr[m]
        ecrow, edrow = crow[m], drow[m]
        cell = ec * NT + eb
        order = np.argsort(cell, kind="stable")
        cell_s = cell[order]
        starts = np.searchsorted(cell_s, np.arange(NCHUNK * NT))
        tpos = np.arange(len(cell_s)) - starts[cell_s]
        g_loc = base.reshape(-1)[cell_s] + tpos // 128
        G = gmap[cell_s // NT, g_loc]
        row = G * 128 + tpos % 128
        g_rows = np.zeros(M, np.int16)
        g_rows[row] = ecrow[order]
        S3 = np.zeros((128, Gtot, 128), FP8)
        S3[tpos % 128, G, edrow[order]] = 1.0
        gidx.append(_wrap16(g_rows))
        Ss.append(np.ascontiguousarray(S3.reshape(128, Gtot * 128)))
    return gidx, Ss, calls, block_groups, need_call, M, Gtot, dinv


def _rep(v, p=128):
    return np.ascontiguousarray(
        np.broadcast_to(np.asarray(v, np.float32), (p, len(v))))


MP_BUFS = 8


def _check_liveness(calls, block_groups, need_call):
    """Every matmul must read a msg tile within the last MP_BUFS of its
    chunk's pool tag, else pool rotation clobbers it.  Emission follows
    the GAH-block lookahead used by layer_loop."""
    emitted = {c: [] for c in range(NCHUNK)}
    ci = 0
    for b in range(NT):
        while ci <= need_call[min(b + GAH, NT - 1)]:
            emitted[calls[ci][0]].append(ci)
            ci += 1
        for (cj, slot, G) in block_groups[b]:
            c = calls[cj][0]
            assert cj in emitted[c][-MP_BUFS:], (
                f"block {b} reads call {cj} beyond pool depth "
                f"{emitted[c][-MP_BUFS - 2:]}")


# ---------------------------------------------------------------- builder

def _build(M, Gtot, calls, block_groups, need_call,
           use_brow1, use_brow2, use_fg, use_fb):
    _check_liveness(calls, block_groups, need_call)
    nc = bacc.Bacc(None, target_bir_lowering=False, num_swdge_queues=NQ)

    def param(name, shape, dt, out=False):
        return nc.declare_dram_parameter(name, shape, dt, isOutput=out)

    u0_own = param("u0_own", [NPAD, D_IN], BF)
    gidx = param("gidx", [128, M // 16], I16)
    S_p = param("S", [128, Gtot * 128], F8)
    dinv_p = param("dinv", [128, NT], F32)
    ident_p = param("ident", [128, 128], BF)
    w0_p = param("w0", [128, D_H], BF)
    w1_p = param("w1", [2, 128, D_H], BF)
    w2_p = param("w2", [2, 128, D_H], BF)
    w3_p = param("w3", [2, 128, D_OUT], BF)
    b0_p = param("b0r", [128, D_H], F32)
    b1_p = param("b1r", [128, D_H], F32)
    b2_p = param("b2r", [128, D_H], F32)
    b3_p = param("b3r", [128, D_OUT], F32)
    brow1_p = param("brow1r", [128, D_H], F32) if use_brow1 else None
    brow2_p = param("brow2r", [128, D_H], F32) if use_brow2 else None
    fg_p = param("fgr", [128, D_OUT], F32) if use_fg else None
    fb_p = param("fbr", [128, D_OUT], F32) if use_fb else None
    out_p = param("out", [NLOC, D_OUT], F32, out=True)

    ul0 = nc.dram_tensor("ul0", [NPAD, D_IN], BF)
    ul23_bf = nc.dram_tensor("ul23_bf", [NPAD, D_H], BF)   # self-term source
    ul23_f8 = nc.dram_tensor("ul23_f8", [NPAD, D_H], F8)   # wire source
    ul4 = nc.dram_tensor("ul4", [NPAD, D_OUT], BF)
    uf0 = [nc.dram_tensor(f"uf0{h}", [NGH, D_IN], BF, addr_space="Shared")
           for h in "AB"]
    uf2 = [nc.dram_tensor(f"uf2{h}", [NGH, D_H], F8, addr_space="Shared")
           for h in "AB"]
    uf3 = [nc.dram_tensor(f"uf3{h}", [NGH, D_H], F8, addr_space="Shared")
           for h in "AB"]
    uf4 = [nc.dram_tensor(f"uf4{h}", [NGH, D_OUT], BF, addr_space="Shared")
           for h in "AB"]

    AX = mybir.AxisListType.X
    AF = mybir.ActivationFunctionType
    OP = mybir.AluOpType

    with tile.TileContext(nc) as tc:
        with (
            tc.tile_pool(name="const", bufs=1) as cp,
            tc.tile_pool(name="hbuf", bufs=1) as hp,
            tc.tile_pool(name="work", bufs=4) as wp,
            tc.tile_pool(name="sqp", bufs=2) as qp,
            tc.tile_pool(name="pair", bufs=4) as pp,
            tc.tile_pool(name="vtp", bufs=VT_AHEAD + 2) as vp,
            tc.tile_pool(name="small", bufs=6) as sp,
            tc.tile_pool(name="msg", bufs=MP_BUFS) as mp,
            tc.tile_pool(name="psT", bufs=2, space="PSUM") as pT,
            tc.tile_pool(name="psM", bufs=2, space="PSUM") as pM,
            tc.tile_pool(name="psA", bufs=4, space="PSUM") as pA,
        ):
            def cload(par, shape, dt):
                t = cp.tile(shape, dt, tag=par.name)
                nc.sync.dma_start(t[:], par[:])
                return t

            def allgather_half(ul, uf, h):
                if SKIP_CC:
                    return
                nc.gpsimd.collective_compute(
                    "AllGather", OP.bypass,
                    ins=[ul[h * HALF:(h + 1) * HALF, :].opt()],
                    outs=[uf[h][:].opt()],
                    replica_groups=[list(range(NCORES))],
                )

            QTR = HALF // 2

            def allgather_b_quarter(ul, uf, q):
                """Quarter q of the second shard-half; output rows are
                rank-strided slices of uf[1] (rank r rows at r*HALF+q*QTR)."""
                if SKIP_CC:
                    return
                d = ul.shape[1]
                ufv = uf[1].rearrange("(r q x) d -> r q (x d)", r=NCORES, q=2)
                nc.gpsimd.collective_compute(
                    "AllGather", OP.bypass,
                    ins=[ul[HALF + q * QTR:HALF + (q + 1) * QTR, :].opt()],
                    outs=[ufv[:, q, :].opt()],
                    replica_groups=[list(range(NCORES))],
                )

            # Layer-1 collectives first (input staged to ul0: collectives
            # can't read IO tensors); they overlap the constant loads below.
            nc.sync.dma_start(ul0[0:HALF, :], u0_own[0:HALF, :])
            allgather_half(ul0, uf0, 0)
            nc.sync.dma_start(ul0[HALF:NPAD, :], u0_own[HALF:NPAD, :])
            allgather_half(ul0, uf0, 1)

            gi = cload(gidx, [128, M // 16], I16)
            S_sb = cload(S_p, [128, Gtot * 128], F8)
            S_v = S_sb.rearrange("p (g d) -> p g d", d=128)
            dv = cload(dinv_p, [128, NT], F32)
            idn = cload(ident_p, [128, 128], BF)
            w0 = cload(w0_p, [128, D_H], BF)

            def wload(par, d):
                t = cp.tile([128, 2, d], BF, tag=par.name)
                nc.sync.dma_start(t[:], par.rearrange("k p d -> p k d"))
                return t

            w1 = wload(w1_p, D_H)
            w2 = wload(w2_p, D_H)
            w3 = wload(w3_p, D_OUT)
            b0 = cload(b0_p, [128, D_H], F32)
            b1 = cload(b1_p, [128, D_H], F32)
            b2 = cload(b2_p, [128, D_H], F32)
            b3 = cload(b3_p, [128, D_OUT], F32)
            brow1 = cload(brow1_p, [128, D_H], F32) if use_brow1 else None
            brow2 = cload(brow2_p, [128, D_H], F32) if use_brow2 else None
            fg = cload(fg_p, [128, D_OUT], F32) if use_fg else None
            fb = cload(fb_p, [128, D_OUT], F32) if use_fb else None

            h_sb = hp.tile([128, NT, D_H], BF)

            def r3(t, d):
                return t.rearrange("(n p) d -> p n d", p=128)

            def emit_gather(ci, uf, d, dt):
                """One gather call -> flat msg tile; returns [128,G,d] view."""
                (c, off, n) = calls[ci]
                t = mp.tile([128, CALLG * d], dt, tag=f"m{c}")
                tv = t.rearrange("p (g d) -> p g d", d=d)
                src = uf[c // 2][(c % 2) * CHUNK:(c % 2 + 1) * CHUNK, :]
                nc.gpsimd.dma_gather(
                    tv[:, : n // 128, :], src,
                    gi[:, off // 16:(off + n) // 16], n, n, d,
                    queue_num=ci % NQ,
                )
                return tv

            def agg_block(b, tiles, vt, d):
                """Self-term seed + one-hot matmuls for block b -> psum."""
                ps = pA.tile([128, d], F32, tag="agg")
                groups = [] if SKIP_AGG else block_groups[b]
                nc.tensor.matmul(ps[:], idn[:], vt[:],
                                 start=True, stop=not groups)
                for j, (ci, slot, G) in enumerate(groups):
                    nc.tensor.matmul(ps[:], S_v[:, G, :], tiles[ci][:, slot, :],
                                     start=False, stop=(j == len(groups) - 1))
                return ps

            def transpose_mm(z_bf, w, d_out, kchunks, evac):
                """z_bf [128, kchunks*128] -> mm = z^T @ w in PSUM.
                evac: 'S' or 'V' engine for the transpose evacuation."""
                mm = pM.tile([128, d_out], F32, tag="mm")
                zt_ps = pT.tile([128, kchunks, 128], BF, tag="zt_ps")
                for kk in range(kchunks):
                    nc.tensor.transpose(
                        zt_ps[:, kk, :], z_bf[:, kk * 128:(kk + 1) * 128],
                        idn[:])
                zt = wp.tile([128, kchunks, 128], BF, tag="zt")
                if evac == "S":
                    nc.scalar.activation(zt[:], zt_ps[:], AF.Copy)
                else:
                    nc.vector.tensor_scalar_mul(zt[:], zt_ps[:], 1.0)
                for kk in range(kchunks):
                    nc.tensor.matmul(mm[:], zt[:, kk, :],
                                     w[:, kk, :] if kchunks > 1 else w[:],
                                     start=(kk == 0), stop=(kk == kchunks - 1))
                return mm

            # ---------------- LN stats over a block pair ------------------
            def pair_stats(p, st):
                """Consumes st['sums'] [128,2]; fills negmu/s/negmu_s [128,2].
                Variance via Square+accum per block (bias = per-block
                negmu), then batched sqrt/recip/scale over the pair."""
                d = st["d"]
                negmu = sp.tile([128, 2], F32, tag="negmu")
                nc.vector.tensor_scalar_mul(negmu[:], st["sums"][:], -1.0 / d)
                ssq = sp.tile([128, 2], F32, tag="ssq")
                for k in (0, 1):
                    sq = qp.tile([128, d], BF, tag="sq")
                    nc.scalar.activation(sq[:], st["h"][k], AF.Square,
                                         bias=negmu[:, k:k + 1],
                                         accum_out=ssq[:, k:k + 1])
                varp = sp.tile([128, 2], F32, tag="varp")
                nc.vector.tensor_scalar(varp[:], ssq[:], 1.0 / d, EPS,
                                        OP.mult, OP.add)
                sd = sp.tile([128, 2], F32, tag="sd")
                nc.scalar.sqrt(sd[:], varp[:])
                rstd = sp.tile([128, 2], F32, tag="rstd")
                nc.vector.reciprocal(rstd[:], sd[:])
                st["negmu"] = negmu
                st["rstd"] = rstd
                return negmu, rstd

            def ln_scale_pair(p, st):
                """s = rstd*dinv, negmu_s = negmu*s for the pair."""
                s = sp.tile([128, 2], F32, tag="s")
                nc.vector.tensor_tensor(s[:], st["rstd"][:],
                                        dv[:, 2 * p:2 * p + 2], OP.mult)
                negmu_s = sp.tile([128, 2], F32, tag="negmu_s")
                nc.vector.tensor_tensor(negmu_s[:], st["negmu"][:], s[:],
                                        OP.mult)
                return s, negmu_s

            def store_u_pair(p, w, brow, ul_bf, zs):
                """z (bf16) per block -> mm -> u bf16+fp8, pair-batched DMA."""
                ubf = pp.tile([128, 2, D_H], BF, tag="ubf")
                u8 = pp.tile([128, 2, D_H], F8, tag="u8")
                for k in (0, 1):
                    mm = transpose_mm(zs[k], w, D_H, 2, evac="S")
                    if brow is not None:
                        b = 2 * p + k
                        nc.vector.scalar_tensor_tensor(
                            ubf[:, k, :], brow[:], dv[:, b:b + 1], mm[:],
                            OP.mult, OP.add)
                    else:
                        nc.vector.tensor_scalar_mul(ubf[:, k, :], mm[:], 1.0)
                    nc.vector.tensor_scalar_mul(u8[:, k, :], ubf[:, k, :],
                                                1.0)
                nc.sync.dma_start(r3(ul_bf, D_H)[:, 2 * p:2 * p + 2, :],
                                  ubf[:])
                nc.sync.dma_start(r3(ul23_f8, D_H)[:, 2 * p:2 * p + 2, :],
                                  u8[:])

            # ---------------- pipelined layer loop ------------------------
            def layer_loop(uf, d, dt, ul_self, passA, stage1, stage2,
                           cc_map=None):
                """passA(b, ps): immediate PSUM evac (writes h/sums).
                stage1(p)/stage2(p): pass-B pair stages, stage1 lagging
                DELTA blocks behind pass A and stage2 one pair-tick
                behind stage1 (so stage2's PE work never waits on fresh
                stage1 results).  Stages are emitted oldest-first within
                an iteration to keep each engine queue head ready.
                mid_cc fires right after stage2(mid_pair)."""
                tiles = {}
                vts = {}
                ci = 0
                lag2 = DELTA + 2 if stage2 is not None else None
                cc_map = cc_map or {}

                def vt_dma(pv):
                    vt = vp.tile([128, 2, d], BF, tag="vt")
                    nc.sync.dma_start(vt[:],
                                      r3(ul_self, d)[:, 2 * pv:2 * pv + 2, :])
                    vts[pv] = vt

                def stage(fn, b, is_last):
                    if fn is not None and 0 <= b < NT and b % 2 == 1:
                        pr = b // 2
                        fn(pr)
                        if is_last and pr in cc_map:
                            cc_map[pr]()

                for pv in range(min(VT_AHEAD, NBH)):
                    vt_dma(pv)
                for b in range(NT):
                    while ci <= need_call[min(b + GAH, NT - 1)]:
                        tiles[ci] = emit_gather(ci, uf, d, dt)
                        ci += 1
                    pv = b // 2 + VT_AHEAD
                    if b % 2 == 0 and pv < NBH:
                        vt_dma(pv)
                    if stage2 is not None:
                        stage(stage2, b - lag2, True)
                        stage(stage1, b - DELTA, False)
                    else:
                        stage(stage1, b - DELTA, True)
                    ps = agg_block(b, tiles, vts[b // 2][:, b % 2, :], d)
                    passA(b, ps)
                end = NT + (lag2 if stage2 is not None else DELTA)
                for b in range(NT, end):
                    if stage2 is not None:
                        stage(stage2, b - lag2, True)
                        stage(stage1, b - DELTA, False)
                    else:
                        stage(stage1, b - DELTA, True)

            # ================= Layer 1 ====================================
            l1_st = {}

            def l1_passA(b, ps):
                p = b // 2
                if b % 2 == 0:
                    l1_st[p] = {"d": D_H,
                                "tbf": pp.tile([128, 2, D_IN], BF,
                                               tag="tbf", name="tbf")}
                nc.scalar.activation(l1_st[p]["tbf"][:, b % 2, :], ps[:],
                                     AF.Copy)

            l1_zs = {}

            def l1_stage1(p):
                st = l1_st.pop(p)
                st["sums"] = sp.tile([128, 2], F32, tag="sums",
                                     name="sums")
                st["h"] = [h_sb[:, 2 * p, :], h_sb[:, 2 * p + 1, :]]
                for k in (0, 1):
                    b = 2 * p + k
                    mm = transpose_mm(st["tbf"][:, k, :], w0, D_H, 1,
                                      evac="V")
                    t2 = wp.tile([128, D_H], F32, tag="t2")
                    nc.vector.scalar_tensor_tensor(
                        t2[:], mm[:], dv[:, b:b + 1], b0[:], OP.mult, OP.add)
                    # relu with row-sum accum, on vector (op1 bypass)
                    nc.vector.scalar_tensor_tensor(
                        h_sb[:, b, :], t2[:], 0.0, t2[:], OP.max, OP.bypass,
                        accum_out=st["sums"][:, k:k + 1])
                pair_stats(p, st)
                s, negmu_s = ln_scale_pair(p, st)
                zs = []
                for k in (0, 1):
                    z = wp.tile([128, D_H], BF, tag="z")
                    nc.scalar.activation(z[:], st["h"][k], AF.Identity,
                                         bias=negmu_s[:, k:k + 1],
                                         scale=s[:, k:k + 1])
                    zs.append(z)
                l1_zs[p] = zs

            def l1_stage2(p):
                store_u_pair(p, w1, brow1, ul23_bf, l1_zs.pop(p))

            layer_loop(uf0, D_IN, BF, u0_own, l1_passA,
                       lambda p: (l1_stage1(p), l1_stage2(p)), None,
                       cc_map={
                           NBH // 2: lambda: allgather_half(ul23_f8, uf2, 0)})
            allgather_half(ul23_f8, uf2, 1)

            # ================= Layers 2, 3 ================================
            def mk_mid(bias, st_map):
                def passA(b, ps):
                    p = b // 2
                    if b % 2 == 0:
                        st_map[p] = {"d": D_H,
                                     "sums": sp.tile([128, 2], F32,
                                                     tag="sums",
                                                     name="sums"),
                                     "h": [h_sb[:, 2 * p, :],
                                           h_sb[:, 2 * p + 1, :]]}
                    st = st_map[p]
                    t4 = wp.tile([128, D_H], F32, tag="t4")
                    nc.vector.scalar_tensor_tensor(
                        t4[:], ps[:], dv[:, b:b + 1], bias[:],
                        OP.mult, OP.add)
                    # h = relu(t4) + h_old, accumulating LN row sums
                    nc.vector.scalar_tensor_tensor(
                        h_sb[:, b, :], t4[:], 0.0, h_sb[:, b, :],
                        OP.max, OP.add, accum_out=st["sums"][:, b % 2:
                                                             b % 2 + 1])
                return passA

            def mk_mid_stage1(st_map, zs_map):
                def stage1(p):
                    st = st_map.pop(p)
                    pair_stats(p, st)
                    s, negmu_s = ln_scale_pair(p, st)
                    zs = []
                    for k in (0, 1):
                        z = wp.tile([128, D_H], BF, tag="z")
                        nc.scalar.activation(z[:], st["h"][k], AF.Identity,
                                             bias=negmu_s[:, k:k + 1],
                                             scale=s[:, k:k + 1])
                        zs.append(z)
                    zs_map[p] = zs
                return stage1

            def mk_mid_stage2(zs_map, w, brow):
                def stage2(p):
                    store_u_pair(p, w, brow, ul23_bf, zs_map.pop(p))
                return stage2

            l2_st, l2_zs = {}, {}
            _l2s1 = mk_mid_stage1(l2_st, l2_zs)
            _l2s2 = mk_mid_stage2(l2_zs, w2, brow2)
            layer_loop(uf2, D_H, F8, ul23_bf, mk_mid(b1, l2_st),
                       lambda p: (_l2s1(p), _l2s2(p)), None,
                       cc_map={
                           NBH // 2: lambda: allgather_half(ul23_f8, uf3, 0)})
            allgather_half(ul23_f8, uf3, 1)

            l3_st, l3_zs = {}, {}

            def l3_stage1(p):
                # u-compute for layer 4: (h * dinv) @ W3  (no LN)
                st = l3_st.pop(p)
                zs = []
                for k in (0, 1):
                    b = 2 * p + k
                    z = wp.tile([128, D_H], BF, tag="z")
                    nc.vector.tensor_scalar_mul(z[:], st["h"][k],
                                                dv[:, b:b + 1])
                    zs.append(z)
                l3_zs[p] = zs

            def l3_stage2(p):
                zs = l3_zs.pop(p)
                ubf = pp.tile([128, 2, D_OUT], BF, tag="ub4")
                for k in (0, 1):
                    mm = transpose_mm(zs[k], w3, D_OUT, 2, evac="S")
                    nc.scalar.activation(ubf[:, k, :], mm[:], AF.Copy)
                nc.sync.dma_start(r3(ul4, D_OUT)[:, 2 * p:2 * p + 2, :],
                                  ubf[:])

            layer_loop(uf3, D_H, F8, ul23_bf, mk_mid(b2, l3_st),
                       lambda p: (l3_stage1(p), l3_stage2(p)), None,
                       cc_map={
                           NBH // 2: lambda: allgather_half(ul4, uf4, 0)})
            allgather_half(ul4, uf4, 1)

            # ================= Layer 4 ====================================
            l4_st = {}

            def l4_passA(b, ps):
                p = b // 2
                if b % 2 == 0:
                    l4_st[p] = {"d": D_OUT,
                                "sums": sp.tile([128, 2], F32, tag="sums",
                                                name="sums"),
                                "y2": pp.tile([128, 2, D_OUT], F32,
                                              tag="y2", name="y2")}
                st = l4_st[p]
                nc.vector.scalar_tensor_tensor(
                    st["y2"][:, b % 2, :], ps[:], dv[:, b:b + 1], b3[:],
                    OP.mult, OP.add, accum_out=st["sums"][:, b % 2:b % 2 + 1])

            def l4_workP(p):
                st = l4_st.pop(p)
                st["h"] = [st["y2"][:, 0, :], st["y2"][:, 1, :]]
                pair_stats(p, st)
                zo = pp.tile([128, 2, D_OUT], F32, tag="zo")
                for k in (0, 1):
                    nc.vector.tensor_scalar(
                        zo[:, k, :], st["y2"][:, k, :],
                        st["negmu"][:, k:k + 1], st["rstd"][:, k:k + 1],
                        OP.add, OP.mult)
                    if fg is not None:
                        nc.vector.tensor_tensor(zo[:, k, :], zo[:, k, :],
                                                fg[:], OP.mult)
                    if fb is not None:
                        nc.vector.tensor_tensor(zo[:, k, :], zo[:, k, :],
                                                fb[:], OP.add)
                lo = 2 * p * 128
                if lo + 256 <= NLOC:
                    nc.sync.dma_start(
                        out_p[lo:lo + 256, :].rearrange(
                            "(n p) d -> p n d", p=128), zo[:])
                else:
                    for k in (0, 1):
                        l2_ = lo + k * 128
                        nrow = min(128, NLOC - l2_)
                        if nrow > 0:
                            nc.sync.dma_start(out_p[l2_:l2_ + nrow, :],
                                              zo[0:nrow, k, :])

            layer_loop(uf4, D_OUT, BF, ul4, l4_passA, l4_workP, None)

    nc.compile()
    return nc


_CACHE = {}


def kernel(x, edge_index, W0, b0, W1, b1, W2, b2, W3, b3,
           ln0_g, ln0_b, ln1_g, ln1_b, fln_g, fln_b):
    x = np.asarray(x, np.float32)
    edge_index = np.asarray(edge_index)
    (gidx, Ss, calls, block_groups, need_call, M, Gtot,
     dinv) = _prep_graph(edge_index)

    W1f = np.asarray(ln0_g, np.float32)[:, None] * np.asarray(W1, np.float32)
    W2f = np.asarray(ln1_g, np.float32)[:, None] * np.asarray(W2, np.float32)
    brow1 = np.asarray(ln0_b, np.float32) @ np.asarray(W1, np.float32)
    brow2 = np.asarray(ln1_b, np.float32) @ np.asarray(W2, np.float32)
    use_brow1 = bool(np.any(brow1 != 0))
    use_brow2 = bool(np.any(brow2 != 0))
    use_fg = bool(np.any(np.asarray(fln_g) != 1))
    use_fb = bool(np.any(np.asarray(fln_b) != 0))

    key = (M, Gtot, tuple(calls), tuple(need_call),
           tuple(tuple(g) for g in block_groups),
           use_brow1, use_brow2, use_fg, use_fb)
    if key not in _CACHE:
        _CACHE[key] = _build(M, Gtot, calls, block_groups, need_call,
                             use_brow1, use_brow2, use_fg, use_fb)
    nc = _CACHE[key]

    u0 = dinv[:, None].astype(np.float32) * x
    u0p = np.zeros((NCORES, NPAD, D_IN), BF16)
    for r in range(NCORES):
        u0p[r, :NLOC] = u0[r * NLOC:(r + 1) * NLOC]
    dinv_pad = np.zeros((NCORES, NPAD), np.float32)
    for r in range(NCORES):
        dinv_pad[r, :NLOC] = dinv[r * NLOC:(r + 1) * NLOC]

    def chunk2(Wf):
        return np.stack([Wf[0:128], Wf[128:256]]).astype(BF16)

    common = {
        "ident": np.eye(128, dtype=BF16),
        "w0": np.asarray(W0, np.float32).astype(BF16),
        "w1": chunk2(W1f), "w2": chunk2(W2f),
        "w3": chunk2(np.asarray(W3, np.float32)),
        "b0r": _rep(b0), "b1r": _rep(b1), "b2r": _rep(b2), "b3r": _rep(b3),
    }
    if use_brow1:
        common["brow1r"] = _rep(brow1)
    if use_brow2:
        common["brow2r"] = _rep(brow2)
    if use_fg:
        common["fgr"] = _rep(fln_g)
    if use_fb:
        common["fbr"] = _rep(fln_b)

    in_maps = []
    for r in range(NCORES):
        m = dict(common)
        m["u0_own"] = u0p[r]
        m["gidx"] = gidx[r]
        m["S"] = Ss[r]
        m["dinv"] = np.ascontiguousarray(dinv_pad[r].reshape(NT, 128).T)
        in_maps.append(m)

    res = run_bass_kernel_spmd(nc, in_maps, core_ids=list(range(NCORES)))
    out = np.concatenate([res.results[r]["out"] for r in range(NCORES)],
                         axis=0)
    return out.astype(np.float32)
